# revision 16
# baseline (speedup 1.0000x reference)
"""FAENet-style GNN message passing on 8 Trainium2 NeuronCores (Bass/Tile).

Sharding: nodes by graph id (contiguous since `batch` is sorted) -> 8 graphs
per core; edges assigned to the core owning their dst node. Per-layer
cross-core AllGather of the down-projected node features (hd) feeds the
src-side gathers. Scatter-add (segment_sum over dst) runs on the tensor
engine as onehot^T @ msg per 128-node window; src gathers use the SWDGE
dma_gather instruction (int16 indices -> hd table split in two halves).
"""

import os
import sys

import numpy as np

for _p in ("/opt/trn_rl_repo", "/root/.axon_site/_ro/trn_rl_repo"):
    if _p not in sys.path and os.path.isdir(_p):
        sys.path.insert(0, _p)

import ml_dtypes  # noqa: E402

BF16 = ml_dtypes.bfloat16

N, E, H, F, G, C, L = 50000, 800000, 128, 128, 50, 92, 4
N_GRAPHS = 64
CUTOFF = 6.0
EPS = 1e-5
NC_ = 8  # cores
P = 128
GPC = N_GRAPHS // NC_  # graphs per core
GCALL = 8  # max chunks per dma_gather call (1024 idx)
WSUB = 4  # chunks per W-psum group


def _build_host(inputs):
    """All integer/index preprocessing + constant tables, per core."""
    x = np.asarray(inputs["x"], np.float32)
    pos = np.asarray(inputs["pos"], np.float32)
    ei = np.asarray(inputs["edge_index"]).astype(np.int64)
    batch = np.asarray(inputs["batch"]).astype(np.int64)
    src, dst = ei[0], ei[1]

    gstart = np.searchsorted(batch, np.arange(0, N_GRAPHS + 1, GPC))
    ns, ne = gstart[:-1], gstart[1:]
    nk = ne - ns
    NSHARD = int(((nk.max() + P - 1) // P) * P)
    NW = NSHARD // P
    SPLIT = (NW // 2) * P
    LENA, LENB = SPLIT, NSHARD - SPLIT
    assert NC_ * LENA < 32768 and NC_ * LENB < 32768

    core_of_node = np.repeat(np.arange(NC_), nk)
    local_of_node = (np.arange(N) - ns[core_of_node]).astype(np.int64)
    edge_core = core_of_node[dst]

    per_core = []
    qlo_max, qhi_max = 1, 1
    for k in range(NC_):
        em = np.nonzero(edge_core == k)[0]
        s_k, d_k = src[em], dst[em]
        dloc = d_k - ns[k]
        win = dloc // P
        s_core = core_of_node[s_k]
        s_local = local_of_node[s_k]
        lo = s_local < SPLIT
        srow = np.where(lo, s_core * LENA + s_local,
                        s_core * LENB + (s_local - SPLIT))
        order = np.lexsort((~lo, win))
        em, dloc, win, srow, lo = (
            em[order], dloc[order], win[order], srow[order], lo[order])
        nlo = np.bincount(win[lo], minlength=NW)
        nhi = np.bincount(win[~lo], minlength=NW)
        per_core.append((em, dloc, win, srow, lo, nlo, nhi))
        qlo_max = max(qlo_max, int(np.ceil(nlo.max() / P)))
        qhi_max = max(qhi_max, int(np.ceil(nhi.max() / P)))

    QLO, QHI = qlo_max, qhi_max
    NLOC = NW * QLO
    NCHUNK = NW * (QLO + QHI)
    ES = NCHUNK * P

    rel = pos[src] - pos[dst]
    distf = np.sqrt((rel * rel).sum(1) + 1e-12)
    off = np.linspace(0.0, CUTOFF, G).astype(np.float32)
    coeff = -0.5 / (off[1] - off[0]) ** 2
    attr_all = np.exp(coeff * (distf[:, None] - off[None, :]) ** 2).astype(np.float32)

    def call_plan(nchunks):
        calls, c = [], 0
        while c < nchunks:
            n = min(GCALL, nchunks - c)
            calls.append((c, n))
            c += n
        return calls

    calls_lo = call_plan(NW * QLO)
    calls_hi = call_plan(NW * QHI)
    chunk_win = np.concatenate(
        [np.repeat(np.arange(NW), QLO), np.repeat(np.arange(NW), QHI)])

    meta = dict(NSHARD=NSHARD, NW=NW, QLO=QLO, QHI=QHI,
                NCHUNK=NCHUNK, ES=ES, NLOC=NLOC,
                SPLIT=SPLIT, LENA=LENA, LENB=LENB,
                calls_lo=calls_lo, calls_hi=calls_hi, chunk_win=chunk_win)

    in_maps = []
    for k in range(NC_):
        em, dloc, win, srow, lo, nlo, nhi = per_core[k]
        slot = np.full(ES, -1, np.int64)
        sdst = np.full(ES, 255, np.int64)
        stab = np.zeros(ES, np.int64)
        pos_lo, pos_hi = np.nonzero(lo)[0], np.nonzero(~lo)[0]
        ofs_lo = np.concatenate(([0], np.cumsum(nlo)))
        ofs_hi = np.concatenate(([0], np.cumsum(nhi)))
        for w in range(NW):
            a, b = int(ofs_lo[w]), int(ofs_lo[w + 1])
            sl0 = w * QLO * P
            idxs = pos_lo[a:b]
            slot[sl0: sl0 + b - a] = em[idxs]
            sdst[sl0: sl0 + b - a] = dloc[idxs] % P
            stab[sl0: sl0 + b - a] = srow[idxs]
            a, b = int(ofs_hi[w]), int(ofs_hi[w + 1])
            sl0 = (NLOC + w * QHI) * P
            idxs = pos_hi[a:b]
            slot[sl0: sl0 + b - a] = em[idxs]
            sdst[sl0: sl0 + b - a] = dloc[idxs] % P
            stab[sl0: sl0 + b - a] = srow[idxs]
        valid = slot >= 0
        eids = np.where(valid, slot, 0)

        relT = np.where(valid[None, :], rel[eids].T, 0.0).astype(BF16)
        attrT = np.where(valid[None, :], attr_all[eids].T, 0.0).astype(BF16)

        sidx = np.where(valid, stab, 0).astype(np.int64)
        blocks = []
        for base_ch, ncall in calls_lo + [(NLOC + c, n) for c, n in calls_hi]:
            ni = ncall * P
            vv = sidx[base_ch * P: base_ch * P + ni]
            blk = vv.reshape(ni // 16, 16).T.astype(np.int16)
            blocks.append(np.tile(blk, (8, 1)))
        idxcat = np.ascontiguousarray(np.concatenate(blocks, axis=1))

        dstloc = np.ascontiguousarray(
            sdst.reshape(NCHUNK, P).T.astype(np.float32)).astype(BF16)

        bloc = np.full(NSHARD, GPC, np.int64)
        bloc[: nk[k]] = batch[ns[k]: ne[k]] - k * GPC
        boh = np.zeros((NSHARD, GPC), np.float32)
        m = bloc < GPC
        boh[np.nonzero(m)[0], bloc[m]] = 1.0
        bonehot = np.ascontiguousarray(
            boh.reshape(NW, P, GPC).transpose(1, 0, 2)).astype(BF16)
        bonehotT = np.ascontiguousarray(
            boh.reshape(NW, P, GPC).transpose(2, 0, 1)).astype(BF16)
        cnt = np.maximum(np.bincount(bloc[m], minlength=GPC), 1.0).astype(np.float32)
        cnt_inv = np.ascontiguousarray((1.0 / cnt).reshape(GPC, 1))
        cntbo2 = (np.bincount(bloc[m], minlength=GPC).astype(np.float32)
                  * float(np.asarray(inputs["bo2"]).reshape(-1)[0])).reshape(GPC, 1)

        xT = np.zeros((C, NSHARD), np.float32)
        xT[:, : nk[k]] = x[ns[k]: ne[k]].T
        xT = xT.astype(BF16)

        in_maps.append(dict(relT=relT, attrT=attrT, idxcat=idxcat, dstloc=dstloc,
                            bonehot=bonehot, bonehotT=bonehotT, cnt_inv=cnt_inv,
                            cntbo2=np.ascontiguousarray(cntbo2), xT=xT))

    w32 = lambda a: np.ascontiguousarray(np.asarray(a, np.float32))
    wbf = lambda a: np.ascontiguousarray(np.asarray(a, np.float32)).astype(BF16)
    iota = np.arange(P, dtype=np.float32)
    shared = dict(
        We1=wbf(inputs["We1"]),
        We2=wbf(inputs["We2"]),
        We3=wbf(inputs["We3"]),
        be12=w32(np.concatenate([np.asarray(inputs["be1"]),
                                 np.asarray(inputs["be2"])])).reshape(F, 1),
        be3=w32(inputs["be3"]).reshape(F, 1),
        Wnode=wbf(inputs["Wnode"]),
        Wlin=wbf(inputs["Wlin"]),
        Wlin2=wbf(inputs["Wlin2"]),
        bnode=w32(inputs["bnode"]).reshape(H, 1),
        blin=w32(inputs["blin"]).reshape(H, 1),
        blin2=w32(inputs["blin2"]).reshape(H, 1),
        Wgeom=wbf(np.transpose(np.asarray(inputs["Wgeom"], np.float32), (1, 0, 2))),
        Wdown=wbf(np.transpose(np.asarray(inputs["Wdown"], np.float32), (1, 0, 2))),
        Wup=wbf(np.transpose(np.asarray(inputs["Wup"], np.float32), (1, 0, 2))),
        bgeom8=wbf(np.tile(np.asarray(inputs["bgeom"], np.float32),
                           (1, GCALL))[None, :, :]),
        bdown1=wbf(np.asarray(inputs["bdown"], np.float32)[None, :, :]),
        bup=w32(np.asarray(inputs["bup"], np.float32).T),
        gnmsB=w32(np.tile(np.asarray(inputs["gnms"], np.float32)[None, :, :],
                          (GPC, 1, 1))),
        gnwB=w32(np.tile(np.asarray(inputs["gnw"], np.float32)[None, :, :],
                         (GPC, 1, 1))),
        gnbB=w32(np.tile(np.asarray(inputs["gnb"], np.float32)[None, :, :],
                         (P, 1, 1))),
        Wo1=wbf(inputs["Wo1"]),
        bo11=wbf(np.asarray(inputs["bo1"], np.float32)[None, :]),
        Wo2=wbf(inputs["Wo2"]),
        ones1=np.ones((1, P), np.float32).astype(BF16),
        iota8=np.ascontiguousarray(
            np.tile(iota[None, None, :], (P, GCALL, 1))).astype(BF16),
        identity=np.eye(P, dtype=np.float32).astype(BF16),
    )
    for m_ in in_maps:
        m_.update(shared)
    return meta, in_maps, dict(ns=ns, ne=ne, nk=nk)


def _build_program(meta):
    import concourse.bass as bass  # noqa: F401
    import concourse.tile as tile
    from concourse import bacc, library_config, mybir

    dt = mybir.dt
    NSHARD, NW = meta["NSHARD"], meta["NW"]
    NCHUNK, ES, NLOC = meta["NCHUNK"], meta["ES"], meta["NLOC"]
    QLO, QHI = meta["QLO"], meta["QHI"]
    SPLIT, LENA, LENB = meta["SPLIT"], meta["LENA"], meta["LENB"]
    chunk_win = meta["chunk_win"]
    calls = [(c, n, 0) for c, n in meta["calls_lo"]] + [
        (NLOC + c, n, 1) for c, n in meta["calls_hi"]]

    nc = bacc.Bacc("TRN2", target_bir_lowering=False, num_devices=NC_,
                   num_swdge_queues=4)

    def din(name, shape, d=dt.float32):
        return nc.dram_tensor(name, shape, d, kind="ExternalInput")

    relT = din("relT", [3, ES], dt.bfloat16)
    attrT = din("attrT", [G, ES], dt.bfloat16)
    idxcat = din("idxcat", [P, ES // 16], dt.int16)
    dstloc = din("dstloc", [P, NCHUNK], dt.bfloat16)
    bonehot = din("bonehot", [P, NW, GPC], dt.bfloat16)
    bonehotT = din("bonehotT", [GPC, NW, P], dt.bfloat16)
    cnt_inv = din("cnt_inv", [GPC, 1])
    cntbo2 = din("cntbo2", [GPC, 1])
    xT = din("xT", [C, NSHARD], dt.bfloat16)
    We1 = din("We1", [3, 64], dt.bfloat16)
    We2 = din("We2", [G, 64], dt.bfloat16)
    We3 = din("We3", [F, F], dt.bfloat16)
    be12 = din("be12", [F, 1])
    be3 = din("be3", [F, 1])
    Wnode = din("Wnode", [C, H], dt.bfloat16)
    Wlin = din("Wlin", [H, H], dt.bfloat16)
    Wlin2 = din("Wlin2", [H, H], dt.bfloat16)
    bnode = din("bnode", [H, 1])
    blin = din("blin", [H, 1])
    blin2 = din("blin2", [H, 1])
    Wgeom = din("Wgeom", [F, L, F], dt.bfloat16)
    Wdown = din("Wdown", [H, L, F], dt.bfloat16)
    Wup = din("Wup", [F, L, H], dt.bfloat16)
    bgeom8 = din("bgeom8", [1, L, GCALL * F], dt.bfloat16)
    bdown1 = din("bdown1", [1, L, F], dt.bfloat16)
    bup = din("bup", [H, L])
    gnmsB = din("gnmsB", [GPC, L, H])
    gnwB = din("gnwB", [GPC, L, H])
    gnbB = din("gnbB", [P, L, H])
    Wo1 = din("Wo1", [H, 64], dt.bfloat16)
    bo11 = din("bo11", [1, 64], dt.bfloat16)
    Wo2 = din("Wo2", [64, 1], dt.bfloat16)
    ones1 = din("ones1", [1, P], dt.bfloat16)
    iota8 = din("iota8", [P, GCALL, P], dt.bfloat16)
    identity = din("identity", [P, P], dt.bfloat16)

    energy = nc.dram_tensor("energy", [GPC, 1], dt.float32, kind="ExternalOutput")
    debug = bool(int(os.environ.get("KERNEL_DEBUG", "0")))
    if debug:
        dbg_h0 = nc.dram_tensor("dbg_h0", [H, NSHARD], dt.float32, kind="ExternalOutput")
        dbg_eT = nc.dram_tensor("dbg_eT", [P, ES], dt.float32, kind="ExternalOutput")
        dbg_agg = nc.dram_tensor("dbg_agg", [P, NW, F], dt.float32, kind="ExternalOutput")
        dbg_h1 = nc.dram_tensor("dbg_h1", [H, NSHARD], dt.float32, kind="ExternalOutput")
        dbg_hd = nc.dram_tensor("dbg_hd", [NSHARD, H], dt.float32, kind="ExternalOutput")
        dbg_p1 = nc.dram_tensor("dbg_p1", [H, 512], dt.float32, kind="ExternalOutput")
        dbg_t2 = nc.dram_tensor("dbg_t2", [H, 512], dt.float32, kind="ExternalOutput")
        dbg_t1 = nc.dram_tensor("dbg_t1", [H, 512], dt.float32, kind="ExternalOutput")

    SI = mybir.ActivationFunctionType.Silu
    SQT = mybir.ActivationFunctionType.Sqrt
    AL = mybir.AluOpType

    with tile.TileContext(nc) as tc:
        with (
            tc.tile_pool(name="dram", bufs=1, space="DRAM") as dram,
            tc.tile_pool(name="const", bufs=1) as cpool,
            tc.tile_pool(name="big", bufs=1) as bigp,
            tc.tile_pool(name="sb", bufs=3) as sb,
            tc.tile_pool(name="wpool", bufs=8) as wpool,
            tc.tile_pool(name="sb2", bufs=3) as sb2,
            tc.tile_pool(name="gat", bufs=8) as gat,
            tc.tile_pool(name="mps", bufs=3, space="PSUM") as mps,
            tc.tile_pool(name="aggps", bufs=2, space="PSUM") as aggps,
            tc.tile_pool(name="sps", bufs=2, space="PSUM") as sps,
            tc.tile_pool(name="gps", bufs=1, space="PSUM") as gps,
        ):
            with tc.tile_critical():
                nc.gpsimd.load_library(library_config.mlp)

            hd_local = dram.tile([NSHARD, H], dt.bfloat16)
            hd_fullA = dram.tile([NC_ * LENA, H], dt.bfloat16)
            hd_fullB = dram.tile([NC_ * LENB, H], dt.bfloat16)
            eT_dram = dram.tile([P, ES], dt.bfloat16)

            _cn = [0]

            def cload(src, shape, d=dt.float32):
                _cn[0] += 1
                t = cpool.tile(shape, d, name=f"cst{_cn[0]}", tag=f"cst{_cn[0]}")
                nc.sync.dma_start(out=t[:], in_=src)
                return t

            c_We1 = cload(We1[:], [3, 64], dt.bfloat16)
            c_We2 = cload(We2[:], [G, 64], dt.bfloat16)
            c_We3 = cload(We3[:], [F, F], dt.bfloat16)
            c_be12 = cload(be12[:], [F, 1])
            c_be3 = cload(be3[:], [F, 1])
            c_Wnode = cload(Wnode[:], [C, H], dt.bfloat16)
            c_Wlin = cload(Wlin[:], [H, H], dt.bfloat16)
            c_Wlin2 = cload(Wlin2[:], [H, H], dt.bfloat16)
            c_bnode = cload(bnode[:], [H, 1])
            c_blin = cload(blin[:], [H, 1])
            c_blin2 = cload(blin2[:], [H, 1])
            c_Wgeom = cload(Wgeom[:], [F, L, F], dt.bfloat16)
            c_Wdown = cload(Wdown[:], [H, L, F], dt.bfloat16)
            c_Wup = cload(Wup[:], [F, L, H], dt.bfloat16)
            c_bgeom8 = cload(bgeom8[:], [1, L, GCALL * F], dt.bfloat16)
            c_bdown1 = cload(bdown1[:], [1, L, F], dt.bfloat16)
            c_bup = cload(bup[:], [H, L])
            c_gnmsB = cload(gnmsB[:], [GPC, L, H])
            c_gnwB = cload(gnwB[:], [GPC, L, H])
            c_gnbB = cload(gnbB[:], [P, L, H])
            c_Wo1 = cload(Wo1[:], [H, 64], dt.bfloat16)
            c_bo11 = cload(bo11[:], [1, 64], dt.bfloat16)
            c_Wo2 = cload(Wo2[:], [64, 1], dt.bfloat16)
            c_ones1 = cload(ones1[:], [1, P], dt.bfloat16)
            c_iota8 = cload(iota8[:], [P, GCALL, P], dt.bfloat16)
            c_ident = cload(identity[:], [P, P], dt.bfloat16)
            c_cnt_inv = cload(cnt_inv[:], [GPC, 1])
            c_cntbo2 = cload(cntbo2[:], [GPC, 1])
            c_boh = cload(bonehot[:], [P, NW, GPC], dt.bfloat16)
            c_bohT = cload(bonehotT[:], [GPC, NW, P], dt.bfloat16)
            c_dstloc = cload(dstloc[:], [P, NCHUNK], dt.bfloat16)
            c_idx = cload(idxcat[:], [P, ES // 16], dt.int16)

            c_eps = cpool.tile([GPC, 1], dt.float32)
            nc.vector.memset(c_eps[:], EPS)

            sim_silu = bool(int(os.environ.get("KERNEL_SIM_SILU", "0")))
            silu_n = [0]

            def act_silu(out_ap, in_ap, bias=None):
                if not sim_silu:
                    if bias is None:
                        nc.scalar.activation(out_ap, in_ap, SI)
                    else:
                        nc.scalar.activation(out_ap, in_ap, SI, bias=bias)
                    return
                silu_n[0] += 1
                shp = list(in_ap.shape)
                pre = sb.tile(shp, dt.float32, name=f"slp{silu_n[0]}", tag="slp")
                ID = mybir.ActivationFunctionType.Identity
                SG = mybir.ActivationFunctionType.Sigmoid
                if bias is None:
                    nc.scalar.activation(pre[:], in_ap, ID)
                else:
                    nc.scalar.activation(pre[:], in_ap, ID, bias=bias)
                sg = sb.tile(shp, dt.float32, name=f"slg{silu_n[0]}", tag="slg")
                nc.scalar.activation(sg[:], pre[:], SG)
                nc.vector.tensor_mul(out_ap, pre[:], sg[:])

            hT = bigp.tile([H, NSHARD], dt.float32)
            hTb = bigp.tile([H, NSHARD], dt.bfloat16)
            agg_sb = bigp.tile([P, NW, F], dt.bfloat16)
            ctr_sb = bigp.tile([P, NW, F], dt.bfloat16)

            # ============ embedding: h0 = MLP(x) ============
            TN = 512
            for j0 in range(0, NSHARD, TN):
                w = min(TN, NSHARD - j0)
                xt = sb.tile([C, TN], dt.bfloat16, tag="xt")
                nc.sync.dma_start(out=xt[:, :w], in_=xT[:, j0: j0 + w])
                p1 = mps.tile([H, TN], dt.float32, tag="mps")
                nc.tensor.matmul(p1[:, :w], lhsT=c_Wnode[:], rhs=xt[:, :w],
                                 start=True, stop=True)
                t1 = sb.tile([H, TN], dt.bfloat16, tag="t1")
                nc.scalar.activation(t1[:, :w], p1[:, :w],
                                     mybir.ActivationFunctionType.Identity,
                                     bias=c_bnode[:])
                if debug and j0 == 0:
                    dt1_ = sb.tile([H, TN], dt.float32, name="dt1_", tag="dbg")
                    nc.vector.tensor_copy(dt1_[:, :w], t1[:, :w])
                    nc.sync.dma_start(out=dbg_t1[:, :w], in_=dt1_[:, :w])

                p2 = mps.tile([H, TN], dt.float32, tag="mps")
                nc.tensor.matmul(p2[:, :w], lhsT=c_Wlin[:], rhs=t1[:, :w],
                                 start=True, stop=True)
                t2 = sb.tile([H, TN], dt.bfloat16, tag="t1")
                act_silu(t2[:, :w], p2[:, :w], bias=c_blin[:])
                if debug and j0 == 0:
                    dt2_ = sb.tile([H, TN], dt.float32, name="dt2_", tag="dbg")
                    nc.vector.tensor_copy(dt2_[:, :w], t2[:, :w])
                    nc.sync.dma_start(out=dbg_t2[:, :w], in_=dt2_[:, :w])
                p3 = mps.tile([H, TN], dt.float32, tag="mps")
                nc.tensor.matmul(p3[:, :w], lhsT=c_Wlin2[:], rhs=t2[:, :w],
                                 start=True, stop=True)
                act_silu(hT[:, j0: j0 + w], p3[:, :w], bias=c_blin2[:])
                nc.vector.tensor_copy(hTb[:, j0: j0 + w], hT[:, j0: j0 + w])

            # ============ embedding: edge features eT ============
            for j0 in range(0, ES, TN):
                ww = min(TN, ES - j0)
                rt = sb.tile([3, TN], dt.bfloat16, tag="rt")
                nc.sync.dma_start(out=rt[:, :ww], in_=relT[:, j0: j0 + ww])
                at = sb.tile([G, TN], dt.bfloat16, tag="at")
                nc.sync.dma_start(out=at[:, :ww], in_=attrT[:, j0: j0 + ww])
                pe = mps.tile([F, TN], dt.float32, tag="mps")
                nc.tensor.matmul(pe[0:64, :ww], lhsT=c_We1[:], rhs=rt[:, :ww],
                                 start=True, stop=True)
                nc.tensor.matmul(pe[64:128, :ww], lhsT=c_We2[:], rhs=at[:, :ww],
                                 start=True, stop=True, tile_position=(0, 64))
                em = sb.tile([F, TN], dt.bfloat16, tag="t1")
                act_silu(em[:, :ww], pe[:, :ww], bias=c_be12[:])
                pf = mps.tile([F, TN], dt.float32, tag="mps")
                nc.tensor.matmul(pf[:, :ww], lhsT=c_We3[:], rhs=em[:, :ww],
                                 start=True, stop=True)
                et = sb.tile([F, TN], dt.bfloat16, tag="t1")
                act_silu(et[:, :ww], pf[:, :ww], bias=c_be3[:])
                nc.sync.dma_start(out=eT_dram[:, j0: j0 + ww], in_=et[:, :ww])

            if debug:
                for j0 in range(0, NSHARD, TN):
                    w = min(TN, NSHARD - j0)
                    dtt = sb.tile([H, TN], dt.float32, name=f"dt{j0}", tag="dbg")
                    nc.vector.tensor_copy(dtt[:, :w], hT[:, j0: j0 + w])
                    nc.sync.dma_start(out=dbg_h0[:, j0: j0 + w], in_=dtt[:, :w])
                for j0 in range(0, ES, TN):
                    ww = min(TN, ES - j0)
                    dte = sb.tile([P, TN], dt.float32, name=f"de{j0}", tag="dbg")
                    dts = sb.tile([P, TN], dt.bfloat16, name=f"ds{j0}", tag="dbg2")
                    nc.sync.dma_start(out=dts[:, :ww], in_=eT_dram[:, j0: j0 + ww])
                    nc.vector.tensor_copy(dte[:, :ww], dts[:, :ww])
                    nc.sync.dma_start(out=dbg_eT[:, j0: j0 + ww], in_=dte[:, :ww])

            # ============ layers ============
            for l in range(L):
                # ---- node phase: hd = silu(h @ Wdown + bdown) -> allgather
                nwa = SPLIT // P
                for w0 in range(0, NW, 4):
                    nwin = min(4, NW - w0)
                    hdt = sb.tile([P, 4, F], dt.bfloat16, tag="hd4")
                    for a in range(nwin):
                        w = w0 + a
                        php = sps.tile([P, F], dt.float32, tag="sps")
                        nc.tensor.matmul(php[:], lhsT=c_ones1[:],
                                         rhs=c_bdown1[:, l, :], start=True, stop=False)
                        nc.tensor.matmul(php[:], lhsT=hTb[:, w * P:(w + 1) * P],
                                         rhs=c_Wdown[:, l, :], start=False, stop=True)
                        act_silu(hdt[:, a, :], php[:])
                    nc.sync.dma_start(
                        out=hd_local[:].rearrange("(a p) d -> p a d", p=P)[
                            :, w0: w0 + nwin, :],
                        in_=hdt[:, :nwin, :])
                    if w0 + 4 >= nwa and w0 < nwa:
                        nc.gpsimd.collective_compute(
                            "AllGather", AL.bypass,
                            replica_groups=[list(range(NC_))],
                            ins=[hd_local[0:SPLIT, :].opt()],
                            outs=[hd_fullA[:].opt()])
                nc.gpsimd.collective_compute(
                    "AllGather", AL.bypass,
                    replica_groups=[list(range(NC_))],
                    ins=[hd_local[SPLIT:, :].opt()], outs=[hd_fullB[:].opt()])

                # ---- edge phase ----
                agg_open = {}
                for call_i, (base_ch, ncall, half) in enumerate(calls):
                    gt = gat.tile([P, GCALL, F], dt.bfloat16, tag="hdg")
                    ni = ncall * P
                    nc.gpsimd.dma_gather(
                        gt[:, :ncall, :],
                        (hd_fullB if half else hd_fullA)[:],
                        c_idx[:, base_ch * 8: base_ch * 8 + ni // 16],
                        ni, ni, F, queue_num=call_i % 4)
                    eTt = wpool.tile([P, GCALL, F], dt.bfloat16, tag="eTt", bufs=4)
                    nc.sync.dma_start(
                        out=eTt[:, :ncall, :],
                        in_=eT_dram[:, base_ch * P: base_ch * P + ni].rearrange(
                            "p (c q) -> p c q", q=P))
                    for s0 in range(0, ncall, WSUB):
                        nsub = min(WSUB, ncall - s0)
                        wp = mps.tile([P, WSUB, F], dt.float32, tag="mps")
                        nc.tensor.matmul(
                            wp[:, :nsub, :].rearrange("p a q -> p (a q)"),
                            lhsT=c_ones1[:],
                            rhs=c_bgeom8[:, l, : nsub * F],
                            start=True, stop=False)
                        for ci in range(nsub):
                            nc.tensor.matmul(
                                wp[:, ci, :], lhsT=eTt[:, s0 + ci, :],
                                rhs=c_Wgeom[:, l, :], start=False, stop=(ci == nsub - 1))
                        wsb = wpool.tile([P, WSUB, F], dt.bfloat16, tag="wsb", bufs=16)
                        act_silu(wsb[:, :nsub, :], wp[:, :nsub, :])
                        msg = sb.tile([P, WSUB, F], dt.bfloat16, tag="msg")
                        nc.vector.tensor_mul(msg[:, :nsub, :], wsb[:, :nsub, :],
                                             gt[:, s0: s0 + nsub, :])
                        oh = sb.tile([P, WSUB, F], dt.bfloat16, tag="oh")
                        nc.vector.tensor_tensor(
                            out=oh[:, :nsub, :],
                            in0=c_iota8[:, :nsub, :],
                            in1=c_dstloc[:, base_ch + s0: base_ch + s0 + nsub
                                         ].to_broadcast([P, nsub, P]),
                            op=AL.is_equal)
                        for ci in range(nsub):
                            ch = base_ch + s0 + ci
                            w = int(chunk_win[ch])
                            in_lo = ch < NLOC
                            q = QLO if in_lo else QHI
                            rel_c = ch - (0 if in_lo else NLOC)
                            first = rel_c % q == 0
                            last = rel_c % q == q - 1
                            key = (w, in_lo)
                            if first:
                                agg_open[key] = aggps.tile(
                                    [P, F], dt.float32, tag="aggps",
                                    name=f"aggp_{l}_{ch}")
                            nc.tensor.matmul(agg_open[key][:], lhsT=oh[:, ci, :],
                                             rhs=msg[:, ci, :],
                                             start=first, stop=last)
                            if last:
                                if in_lo:
                                    nc.vector.tensor_copy(agg_sb[:, w, :],
                                                          agg_open[key][:])
                                else:
                                    nc.vector.tensor_add(agg_sb[:, w, :],
                                                         agg_sb[:, w, :],
                                                         agg_open[key][:])
                                del agg_open[key]

                if debug and l == 0:
                    for w in range(NW):
                        dta = sb.tile([P, F], dt.float32, name=f"da{w}", tag="dbg")
                        nc.vector.tensor_copy(dta[:], agg_sb[:, w, :])
                        nc.sync.dma_start(
                            out=dbg_agg[:, w, :], in_=dta[:])
                    for w in range(NW):
                        dhs = sb.tile([P, F], dt.bfloat16, name=f"dq{w}", tag="dbg2")
                        nc.sync.dma_start(
                            out=dhs[:],
                            in_=hd_local[:].rearrange("(a p) d -> p a d", p=P)[:, w, :])
                        dth = sb.tile([P, F], dt.float32, name=f"dh{w}", tag="dbg")
                        nc.vector.tensor_copy(dth[:], dhs[:])
                        nc.sync.dma_start(
                            out=dbg_hd.rearrange("(a p) d -> p a d", p=P)[:, w, :],
                            in_=dth[:])

                # ---- GraphNorm + update ----
                gsum_p = gps.tile([GPC, H], dt.float32, tag="gps")
                for w in range(NW):
                    nc.tensor.matmul(gsum_p[:], lhsT=c_boh[:, w, :],
                                     rhs=agg_sb[:, w, :],
                                     start=(w == 0), stop=(w == NW - 1))
                tmean = sb2.tile([GPC, H], dt.float32, tag="gn32")
                nc.vector.tensor_scalar(out=tmean[:], in0=gsum_p[:],
                                        scalar1=c_cnt_inv[:], scalar2=None,
                                        op0=AL.mult)
                mean_sc = sb2.tile([GPC, H], dt.bfloat16, tag="gn")
                nc.vector.tensor_mul(mean_sc[:], tmean[:], c_gnmsB[:, l, :])
                for w in range(NW):
                    mb = sps.tile([P, H], dt.float32, tag="sps")
                    nc.tensor.matmul(mb[:], lhsT=c_bohT[:, w, :], rhs=mean_sc[:],
                                     start=True, stop=True)
                    nc.vector.tensor_sub(ctr_sb[:, w, :], agg_sb[:, w, :], mb[:])
                sq_p = gps.tile([GPC, H], dt.float32, tag="gps")
                for w in range(NW):
                    sq = sb2.tile([P, H], dt.bfloat16, tag="sq")
                    nc.vector.tensor_mul(sq[:], ctr_sb[:, w, :], ctr_sb[:, w, :])
                    nc.tensor.matmul(sq_p[:], lhsT=c_boh[:, w, :], rhs=sq[:],
                                     start=(w == 0), stop=(w == NW - 1))
                var = sb2.tile([GPC, H], dt.float32, tag="gn32")
                nc.vector.tensor_scalar(out=var[:], in0=sq_p[:],
                                        scalar1=c_cnt_inv[:], scalar2=None,
                                        op0=AL.mult)
                sd = sb2.tile([GPC, H], dt.float32, tag="gn32")
                nc.scalar.activation(sd[:], var[:], SQT, bias=c_eps[:])
                rs = sb2.tile([GPC, H], dt.float32, tag="gn32")
                nc.vector.reciprocal(rs[:], sd[:])
                scale = sb2.tile([GPC, H], dt.bfloat16, tag="gn")
                nc.vector.tensor_mul(scale[:], rs[:], c_gnwB[:, l, :])
                for w0 in range(0, NW, 4):
                    nwin = min(4, NW - w0)
                    hnT4 = sb2.tile([F, 4 * P], dt.bfloat16, tag="hnT4")
                    for a in range(nwin):
                        w = w0 + a
                        sbp = sps.tile([P, H], dt.float32, tag="sps")
                        nc.tensor.matmul(sbp[:], lhsT=c_bohT[:, w, :], rhs=scale[:],
                                         start=True, stop=True)
                        hn = sb2.tile([P, H], dt.float32, tag="hn")
                        nc.vector.tensor_mul(hn[:], ctr_sb[:, w, :], sbp[:])
                        hn2 = sb2.tile([P, H], dt.float32, tag="hn2")
                        nc.vector.tensor_add(hn2[:], hn[:], c_gnbB[:, l, :])
                        shn = sb2.tile([P, H], dt.bfloat16, tag="shn")
                        act_silu(shn[:], hn2[:])
                        tp = sps.tile([P, P], dt.bfloat16, tag="sps")
                        nc.tensor.transpose(tp[:], shn[:], c_ident[:])
                        nc.vector.tensor_copy(hnT4[:, a * P:(a + 1) * P], tp[:])
                    upp = mps.tile([H, 4 * P], dt.float32, tag="mps")
                    nc.tensor.matmul(upp[:, : nwin * P], lhsT=c_Wup[:, l, :],
                                     rhs=hnT4[:, : nwin * P], start=True, stop=True)
                    ups = sb2.tile([H, 4 * P], dt.float32, tag="ups")
                    act_silu(ups[:, : nwin * P], upp[:, : nwin * P], bias=c_bup[:, l: l + 1])
                    nc.vector.tensor_add(hT[:, w0 * P: w0 * P + nwin * P],
                                         hT[:, w0 * P: w0 * P + nwin * P],
                                         ups[:, : nwin * P])
                    nc.vector.tensor_copy(hTb[:, w0 * P: w0 * P + nwin * P],
                                          hT[:, w0 * P: w0 * P + nwin * P])

                if debug and l == 0:
                    for j0 in range(0, NSHARD, TN):
                        w_ = min(TN, NSHARD - j0)
                        dt1 = sb.tile([H, TN], dt.float32, name=f"d1{j0}", tag="dbg")
                        nc.vector.tensor_copy(dt1[:, :w_], hT[:, j0: j0 + w_])
                        nc.sync.dma_start(out=dbg_h1[:, j0: j0 + w_], in_=dt1[:, :w_])

            # ============ output block ============
            z_p = gps.tile([GPC, 64], dt.float32, tag="gps")
            for w in range(NW):
                t3p = sps.tile([P, 64], dt.float32, tag="sps")
                nc.tensor.matmul(t3p[:], lhsT=c_ones1[:], rhs=c_bo11[:],
                                 start=True, stop=False)
                nc.tensor.matmul(t3p[:], lhsT=hTb[:, w * P:(w + 1) * P],
                                 rhs=c_Wo1[:], start=False, stop=True)
                t3 = sb2.tile([P, 64], dt.bfloat16, tag="t3b")
                act_silu(t3[:], t3p[:])
                nc.tensor.matmul(z_p[:], lhsT=c_boh[:, w, :], rhs=t3[:],
                                 start=(w == 0), stop=(w == NW - 1))
            z_sb = sb2.tile([GPC, 64], dt.bfloat16, tag="zsb")
            nc.vector.tensor_copy(z_sb[:], z_p[:])
            zT_p = sps.tile([64, GPC], dt.bfloat16, tag="sps")
            nc.tensor.transpose(zT_p[:], z_sb[:], c_ident[:GPC, :GPC])
            zT = sb2.tile([64, GPC], dt.bfloat16, tag="zT")
            nc.vector.tensor_copy(zT[:], zT_p[:])
            en_p = sps.tile([GPC, 1], dt.float32, tag="sps")
            nc.tensor.matmul(en_p[:], lhsT=zT[:], rhs=c_Wo2[:],
                             start=True, stop=True)
            en = sb2.tile([GPC, 1], dt.float32, tag="en")
            nc.vector.tensor_add(en[:], en_p[:], c_cntbo2[:])
            nc.sync.dma_start(out=energy[:], in_=en[:])

    nc.compile()
    return nc


def _install_ntff_hook():
    """Restore antenv.axon_hooks + register the ctypes NTFF hook."""
    import types

    try:
        from antenv.axon_hooks import get_axon_ntff_profile_hook  # noqa: F401

        return
    except ImportError:
        pass
    try:
        import antenv

        mod = types.ModuleType("antenv.axon_hooks")
        mod._hook = None

        def _set(h):
            mod._hook = h

        def _get():
            return mod._hook

        mod.set_axon_ntff_profile_hook = _set
        mod.get_axon_ntff_profile_hook = _get
        sys.modules["antenv.axon_hooks"] = mod
        antenv.axon_hooks = mod
        sys.path.insert(0, "/root/.axon_site")
        from trn_agent_boot.trn_boot import _ntff_profile_via_ctypes

        hook = _ntff_profile_via_ctypes("/opt/axon/libaxon_pjrt.so")
        if hook is not None:
            _set(hook)
    except Exception as e:  # pragma: no cover
        print(f"ntff hook install failed: {e}", file=sys.stderr)


def kernel(**inputs) -> np.ndarray:
    meta, in_maps, _extra = _build_host(inputs)
    nc = _build_program(meta)
    from concourse.bass_utils import run_bass_kernel_spmd

    trace = bool(int(os.environ.get("KERNEL_TRACE", "0")))
    if trace:
        _install_ntff_hook()
    res = run_bass_kernel_spmd(nc, in_maps, core_ids=list(range(NC_)), trace=trace)
    if trace:
        kernel.last_results = res
    out = np.concatenate([res.results[k]["energy"] for k in range(NC_)], axis=0)
    return out.astype(np.float32)



# revision 19
# speedup vs baseline: 1.1388x; 1.1388x over previous
"""FAENet-style GNN message passing on 8 Trainium2 NeuronCores (Bass/Tile).

Sharding: nodes by graph id (contiguous since `batch` is sorted) -> 8 graphs
per core; edges assigned to the core owning their dst node. Per-layer
cross-core AllGather of the down-projected node features (hd) feeds the
src-side gathers. Scatter-add (segment_sum over dst) runs on the tensor
engine as onehot^T @ msg per 128-node window; src gathers use the SWDGE
dma_gather instruction (int16 indices -> hd table split in two halves).
"""

import os
import sys

import numpy as np

for _p in ("/opt/trn_rl_repo", "/root/.axon_site/_ro/trn_rl_repo"):
    if _p not in sys.path and os.path.isdir(_p):
        sys.path.insert(0, _p)

import ml_dtypes  # noqa: E402

BF16 = ml_dtypes.bfloat16

N, E, H, F, G, C, L = 50000, 800000, 128, 128, 50, 92, 4
N_GRAPHS = 64
CUTOFF = 6.0
EPS = 1e-5
NC_ = 8  # cores
P = 128
GPC = N_GRAPHS // NC_  # graphs per core
GCALL = 8  # max chunks per dma_gather call (1024 idx)
WSUB = 4  # chunks per W-psum group


def _build_host(inputs):
    """All integer/index preprocessing + constant tables, per core."""
    x = np.asarray(inputs["x"], np.float32)
    pos = np.asarray(inputs["pos"], np.float32)
    ei = np.asarray(inputs["edge_index"]).astype(np.int64)
    batch = np.asarray(inputs["batch"]).astype(np.int64)
    src, dst = ei[0], ei[1]

    gstart = np.searchsorted(batch, np.arange(0, N_GRAPHS + 1, GPC))
    ns, ne = gstart[:-1], gstart[1:]
    nk = ne - ns
    NSHARD = int(((nk.max() + P - 1) // P) * P)
    NW = NSHARD // P
    HALF = (NC_ // 2) * NSHARD
    assert HALF < 32768

    core_of_node = np.repeat(np.arange(NC_), nk)
    trow = (core_of_node * NSHARD + (np.arange(N) - ns[core_of_node])).astype(np.int64)
    edge_core = core_of_node[dst]

    per_core = []
    qlo_max, qhi_max = 1, 1
    for k in range(NC_):
        em = np.nonzero(edge_core == k)[0]
        s_k, d_k = src[em], dst[em]
        dloc = d_k - ns[k]
        win = dloc // P
        srow = trow[s_k]
        lo = srow < HALF
        order = np.lexsort((~lo, win))
        em, dloc, win, srow, lo = (
            em[order], dloc[order], win[order], srow[order], lo[order])
        nlo = np.bincount(win[lo], minlength=NW)
        nhi = np.bincount(win[~lo], minlength=NW)
        per_core.append((em, dloc, win, srow, lo, nlo, nhi))
        qlo_max = max(qlo_max, int(np.ceil(nlo.max() / P)))
        qhi_max = max(qhi_max, int(np.ceil(nhi.max() / P)))

    QLO, QHI = qlo_max, qhi_max
    NLOC = NW * QLO
    NCHUNK = NW * (QLO + QHI)
    ES = NCHUNK * P

    rel = pos[src] - pos[dst]
    distf = np.sqrt((rel * rel).sum(1) + 1e-12)
    off = np.linspace(0.0, CUTOFF, G).astype(np.float32)
    coeff = -0.5 / (off[1] - off[0]) ** 2
    attr_all = np.exp(coeff * (distf[:, None] - off[None, :]) ** 2).astype(np.float32)

    def call_plan(nchunks):
        calls, c = [], 0
        while c < nchunks:
            n = min(GCALL, nchunks - c)
            calls.append((c, n))
            c += n
        return calls

    calls_lo = call_plan(NW * QLO)
    calls_hi = call_plan(NW * QHI)
    chunk_win = np.concatenate(
        [np.repeat(np.arange(NW), QLO), np.repeat(np.arange(NW), QHI)])

    meta = dict(NSHARD=NSHARD, NW=NW, HALF=HALF, QLO=QLO, QHI=QHI,
                NCHUNK=NCHUNK, ES=ES, NLOC=NLOC,
                calls_lo=calls_lo, calls_hi=calls_hi, chunk_win=chunk_win)

    in_maps = []
    for k in range(NC_):
        em, dloc, win, srow, lo, nlo, nhi = per_core[k]
        slot = np.full(ES, -1, np.int64)
        sdst = np.full(ES, 255, np.int64)
        stab = np.zeros(ES, np.int64)
        pos_lo, pos_hi = np.nonzero(lo)[0], np.nonzero(~lo)[0]
        ofs_lo = np.concatenate(([0], np.cumsum(nlo)))
        ofs_hi = np.concatenate(([0], np.cumsum(nhi)))
        for w in range(NW):
            a, b = int(ofs_lo[w]), int(ofs_lo[w + 1])
            sl0 = w * QLO * P
            idxs = pos_lo[a:b]
            slot[sl0: sl0 + b - a] = em[idxs]
            sdst[sl0: sl0 + b - a] = dloc[idxs] % P
            stab[sl0: sl0 + b - a] = srow[idxs]
            a, b = int(ofs_hi[w]), int(ofs_hi[w + 1])
            sl0 = (NLOC + w * QHI) * P
            idxs = pos_hi[a:b]
            slot[sl0: sl0 + b - a] = em[idxs]
            sdst[sl0: sl0 + b - a] = dloc[idxs] % P
            stab[sl0: sl0 + b - a] = srow[idxs] - HALF
        valid = slot >= 0
        eids = np.where(valid, slot, 0)

        relT = np.where(valid[None, :], rel[eids].T, 0.0).astype(BF16)
        attrT = np.where(valid[None, :], attr_all[eids].T, 0.0).astype(BF16)

        sidx = np.where(valid, stab, 0).astype(np.int64)
        blocks = []
        for base_ch, ncall in calls_lo + [(NLOC + c, n) for c, n in calls_hi]:
            ni = ncall * P
            vv = sidx[base_ch * P: base_ch * P + ni]
            blk = vv.reshape(ni // 16, 16).T.astype(np.int16)
            blocks.append(np.tile(blk, (8, 1)))
        idxcat = np.ascontiguousarray(np.concatenate(blocks, axis=1))

        dstloc = np.ascontiguousarray(
            sdst.reshape(NCHUNK, P).T.astype(np.float32)).astype(BF16)

        bloc = np.full(NSHARD, GPC, np.int64)
        bloc[: nk[k]] = batch[ns[k]: ne[k]] - k * GPC
        boh = np.zeros((NSHARD, GPC), np.float32)
        m = bloc < GPC
        boh[np.nonzero(m)[0], bloc[m]] = 1.0
        bonehot = np.ascontiguousarray(
            boh.reshape(NW, P, GPC).transpose(1, 0, 2)).astype(BF16)
        bonehotT = np.ascontiguousarray(
            boh.reshape(NW, P, GPC).transpose(2, 0, 1)).astype(BF16)
        cnt = np.maximum(np.bincount(bloc[m], minlength=GPC), 1.0).astype(np.float32)
        cnt_inv = np.ascontiguousarray((1.0 / cnt).reshape(GPC, 1))
        cntbo2 = (np.bincount(bloc[m], minlength=GPC).astype(np.float32)
                  * float(np.asarray(inputs["bo2"]).reshape(-1)[0])).reshape(GPC, 1)

        xT = np.zeros((C, NSHARD), np.float32)
        xT[:, : nk[k]] = x[ns[k]: ne[k]].T
        xT = xT.astype(BF16)

        in_maps.append(dict(relT=relT, attrT=attrT, idxcat=idxcat, dstloc=dstloc,
                            bonehot=bonehot, bonehotT=bonehotT, cnt_inv=cnt_inv,
                            cntbo2=np.ascontiguousarray(cntbo2), xT=xT))

    w32 = lambda a: np.ascontiguousarray(np.asarray(a, np.float32))
    wbf = lambda a: np.ascontiguousarray(np.asarray(a, np.float32)).astype(BF16)
    iota = np.arange(P, dtype=np.float32)
    shared = dict(
        We1=wbf(inputs["We1"]),
        We2=wbf(inputs["We2"]),
        We3=wbf(inputs["We3"]),
        be12=w32(np.concatenate([np.asarray(inputs["be1"]),
                                 np.asarray(inputs["be2"])])).reshape(F, 1),
        be3=w32(inputs["be3"]).reshape(F, 1),
        Wnode=wbf(inputs["Wnode"]),
        Wlin=wbf(inputs["Wlin"]),
        Wlin2=wbf(inputs["Wlin2"]),
        bnode=w32(inputs["bnode"]).reshape(H, 1),
        blin=w32(inputs["blin"]).reshape(H, 1),
        blin2=w32(inputs["blin2"]).reshape(H, 1),
        Wgeom=wbf(np.transpose(np.asarray(inputs["Wgeom"], np.float32), (1, 0, 2))),
        Wdown=wbf(np.transpose(np.asarray(inputs["Wdown"], np.float32), (1, 0, 2))),
        Wup=wbf(np.transpose(np.asarray(inputs["Wup"], np.float32), (1, 0, 2))),
        bgeom8=wbf(np.tile(np.asarray(inputs["bgeom"], np.float32),
                           (1, GCALL))[None, :, :]),
        bdown1=wbf(np.asarray(inputs["bdown"], np.float32)[None, :, :]),
        bup=w32(np.asarray(inputs["bup"], np.float32).T),
        gnmsB=w32(np.tile(np.asarray(inputs["gnms"], np.float32)[None, :, :],
                          (GPC, 1, 1))),
        gnwB=w32(np.tile(np.asarray(inputs["gnw"], np.float32)[None, :, :],
                         (GPC, 1, 1))),
        gnbB=w32(np.tile(np.asarray(inputs["gnb"], np.float32)[None, :, :],
                         (P, 1, 1))),
        Wo1=wbf(inputs["Wo1"]),
        bo11=wbf(np.asarray(inputs["bo1"], np.float32)[None, :]),
        Wo2=wbf(inputs["Wo2"]),
        ones1=np.ones((1, P), np.float32).astype(BF16),
        iota8=np.ascontiguousarray(
            np.tile(iota[None, None, :], (P, GCALL, 1))).astype(BF16),
        identity=np.eye(P, dtype=np.float32).astype(BF16),
    )
    for m_ in in_maps:
        m_.update(shared)
    return meta, in_maps, dict(ns=ns, ne=ne, nk=nk)


def _build_program(meta):
    import concourse.bass as bass  # noqa: F401
    import concourse.tile as tile
    from concourse import bacc, library_config, mybir

    dt = mybir.dt
    NSHARD, NW = meta["NSHARD"], meta["NW"]
    NCHUNK, ES, NLOC = meta["NCHUNK"], meta["ES"], meta["NLOC"]
    QLO, QHI = meta["QLO"], meta["QHI"]
    chunk_win = meta["chunk_win"]
    calls = [(c, n, 0) for c, n in meta["calls_lo"]] + [
        (NLOC + c, n, 1) for c, n in meta["calls_hi"]]

    nc = bacc.Bacc("TRN2", target_bir_lowering=False, num_devices=NC_,
                   num_swdge_queues=4)

    def din(name, shape, d=dt.float32):
        return nc.dram_tensor(name, shape, d, kind="ExternalInput")

    relT = din("relT", [3, ES], dt.bfloat16)
    attrT = din("attrT", [G, ES], dt.bfloat16)
    idxcat = din("idxcat", [P, ES // 16], dt.int16)
    dstloc = din("dstloc", [P, NCHUNK], dt.bfloat16)
    bonehot = din("bonehot", [P, NW, GPC], dt.bfloat16)
    bonehotT = din("bonehotT", [GPC, NW, P], dt.bfloat16)
    cnt_inv = din("cnt_inv", [GPC, 1])
    cntbo2 = din("cntbo2", [GPC, 1])
    xT = din("xT", [C, NSHARD], dt.bfloat16)
    We1 = din("We1", [3, 64], dt.bfloat16)
    We2 = din("We2", [G, 64], dt.bfloat16)
    We3 = din("We3", [F, F], dt.bfloat16)
    be12 = din("be12", [F, 1])
    be3 = din("be3", [F, 1])
    Wnode = din("Wnode", [C, H], dt.bfloat16)
    Wlin = din("Wlin", [H, H], dt.bfloat16)
    Wlin2 = din("Wlin2", [H, H], dt.bfloat16)
    bnode = din("bnode", [H, 1])
    blin = din("blin", [H, 1])
    blin2 = din("blin2", [H, 1])
    Wgeom = din("Wgeom", [F, L, F], dt.bfloat16)
    Wdown = din("Wdown", [H, L, F], dt.bfloat16)
    Wup = din("Wup", [F, L, H], dt.bfloat16)
    bgeom8 = din("bgeom8", [1, L, GCALL * F], dt.bfloat16)
    bdown1 = din("bdown1", [1, L, F], dt.bfloat16)
    bup = din("bup", [H, L])
    gnmsB = din("gnmsB", [GPC, L, H])
    gnwB = din("gnwB", [GPC, L, H])
    gnbB = din("gnbB", [P, L, H])
    Wo1 = din("Wo1", [H, 64], dt.bfloat16)
    bo11 = din("bo11", [1, 64], dt.bfloat16)
    Wo2 = din("Wo2", [64, 1], dt.bfloat16)
    ones1 = din("ones1", [1, P], dt.bfloat16)
    iota8 = din("iota8", [P, GCALL, P], dt.bfloat16)
    identity = din("identity", [P, P], dt.bfloat16)

    energy = nc.dram_tensor("energy", [GPC, 1], dt.float32, kind="ExternalOutput")
    debug = bool(int(os.environ.get("KERNEL_DEBUG", "0")))
    if debug:
        dbg_h0 = nc.dram_tensor("dbg_h0", [H, NSHARD], dt.float32, kind="ExternalOutput")
        dbg_eT = nc.dram_tensor("dbg_eT", [P, ES], dt.float32, kind="ExternalOutput")
        dbg_agg = nc.dram_tensor("dbg_agg", [P, NW, F], dt.float32, kind="ExternalOutput")
        dbg_h1 = nc.dram_tensor("dbg_h1", [H, NSHARD], dt.float32, kind="ExternalOutput")
        dbg_hd = nc.dram_tensor("dbg_hd", [NSHARD, H], dt.float32, kind="ExternalOutput")
        dbg_p1 = nc.dram_tensor("dbg_p1", [H, 512], dt.float32, kind="ExternalOutput")
        dbg_t2 = nc.dram_tensor("dbg_t2", [H, 512], dt.float32, kind="ExternalOutput")
        dbg_t1 = nc.dram_tensor("dbg_t1", [H, 512], dt.float32, kind="ExternalOutput")

    SI = mybir.ActivationFunctionType.Silu
    SQT = mybir.ActivationFunctionType.Sqrt
    AL = mybir.AluOpType

    with tile.TileContext(nc) as tc:
        with (
            tc.tile_pool(name="dram", bufs=1, space="DRAM") as dram,
            tc.tile_pool(name="const", bufs=1) as cpool,
            tc.tile_pool(name="big", bufs=1) as bigp,
            tc.tile_pool(name="sb", bufs=3) as sb,
            tc.tile_pool(name="wpool", bufs=8) as wpool,
            tc.tile_pool(name="sb2", bufs=3) as sb2,
            tc.tile_pool(name="gat", bufs=12) as gat,
            tc.tile_pool(name="mps", bufs=3, space="PSUM") as mps,
            tc.tile_pool(name="aggps", bufs=2, space="PSUM") as aggps,
            tc.tile_pool(name="sps", bufs=2, space="PSUM") as sps,
            tc.tile_pool(name="gps", bufs=1, space="PSUM") as gps,
        ):
            with tc.tile_critical():
                nc.gpsimd.load_library(library_config.mlp)

            hd_local = dram.tile([NSHARD, H], dt.bfloat16)
            hd_full = nc.dram_tensor(
                "hd_full_sh", [NC_ * NSHARD, H], dt.bfloat16,
                kind="Internal", addr_space="Shared")
            eT_dram = dram.tile([P, ES], dt.bfloat16)

            _cn = [0]

            def cload(src, shape, d=dt.float32):
                _cn[0] += 1
                t = cpool.tile(shape, d, name=f"cst{_cn[0]}", tag=f"cst{_cn[0]}")
                nc.sync.dma_start(out=t[:], in_=src)
                return t

            c_We1 = cload(We1[:], [3, 64], dt.bfloat16)
            c_We2 = cload(We2[:], [G, 64], dt.bfloat16)
            c_We3 = cload(We3[:], [F, F], dt.bfloat16)
            c_be12 = cload(be12[:], [F, 1])
            c_be3 = cload(be3[:], [F, 1])
            c_Wnode = cload(Wnode[:], [C, H], dt.bfloat16)
            c_Wlin = cload(Wlin[:], [H, H], dt.bfloat16)
            c_Wlin2 = cload(Wlin2[:], [H, H], dt.bfloat16)
            c_bnode = cload(bnode[:], [H, 1])
            c_blin = cload(blin[:], [H, 1])
            c_blin2 = cload(blin2[:], [H, 1])
            c_Wgeom = cload(Wgeom[:], [F, L, F], dt.bfloat16)
            c_Wdown = cload(Wdown[:], [H, L, F], dt.bfloat16)
            c_Wup = cload(Wup[:], [F, L, H], dt.bfloat16)
            c_bgeom8 = cload(bgeom8[:], [1, L, GCALL * F], dt.bfloat16)
            c_bdown1 = cload(bdown1[:], [1, L, F], dt.bfloat16)
            c_bup = cload(bup[:], [H, L])
            c_gnmsB = cload(gnmsB[:], [GPC, L, H])
            c_gnwB = cload(gnwB[:], [GPC, L, H])
            c_gnbB = cload(gnbB[:], [P, L, H])
            c_Wo1 = cload(Wo1[:], [H, 64], dt.bfloat16)
            c_bo11 = cload(bo11[:], [1, 64], dt.bfloat16)
            c_Wo2 = cload(Wo2[:], [64, 1], dt.bfloat16)
            c_ones1 = cload(ones1[:], [1, P], dt.bfloat16)
            c_iota8 = cload(iota8[:], [P, GCALL, P], dt.bfloat16)
            c_ident = cload(identity[:], [P, P], dt.bfloat16)
            c_cnt_inv = cload(cnt_inv[:], [GPC, 1])
            c_cntbo2 = cload(cntbo2[:], [GPC, 1])
            c_boh = cload(bonehot[:], [P, NW, GPC], dt.bfloat16)
            c_bohT = cload(bonehotT[:], [GPC, NW, P], dt.bfloat16)
            c_dstloc = cload(dstloc[:], [P, NCHUNK], dt.bfloat16)
            c_idx = cload(idxcat[:], [P, ES // 16], dt.int16)

            c_eps = cpool.tile([GPC, 1], dt.float32)
            nc.vector.memset(c_eps[:], EPS)

            sim_silu = bool(int(os.environ.get("KERNEL_SIM_SILU", "0")))
            silu_n = [0]

            def act_silu(out_ap, in_ap, bias=None):
                if not sim_silu:
                    if bias is None:
                        nc.scalar.activation(out_ap, in_ap, SI)
                    else:
                        nc.scalar.activation(out_ap, in_ap, SI, bias=bias)
                    return
                silu_n[0] += 1
                shp = list(in_ap.shape)
                pre = sb.tile(shp, dt.float32, name=f"slp{silu_n[0]}", tag="slp")
                ID = mybir.ActivationFunctionType.Identity
                SG = mybir.ActivationFunctionType.Sigmoid
                if bias is None:
                    nc.scalar.activation(pre[:], in_ap, ID)
                else:
                    nc.scalar.activation(pre[:], in_ap, ID, bias=bias)
                sg = sb.tile(shp, dt.float32, name=f"slg{silu_n[0]}", tag="slg")
                nc.scalar.activation(sg[:], pre[:], SG)
                nc.vector.tensor_mul(out_ap, pre[:], sg[:])

            hT = bigp.tile([H, NSHARD], dt.float32)
            hTb = bigp.tile([H, NSHARD], dt.bfloat16)
            agg_sb = bigp.tile([P, NW, F], dt.bfloat16)
            ctr_sb = bigp.tile([P, NW, F], dt.bfloat16)

            # ============ embedding: h0 = MLP(x) ============
            TN = 512
            for j0 in range(0, NSHARD, TN):
                w = min(TN, NSHARD - j0)
                xt = sb.tile([C, TN], dt.bfloat16, tag="xt")
                nc.sync.dma_start(out=xt[:, :w], in_=xT[:, j0: j0 + w])
                p1 = mps.tile([H, TN], dt.float32, tag="mps")
                nc.tensor.matmul(p1[:, :w], lhsT=c_Wnode[:], rhs=xt[:, :w],
                                 start=True, stop=True)
                t1 = sb.tile([H, TN], dt.bfloat16, tag="t1")
                nc.scalar.activation(t1[:, :w], p1[:, :w],
                                     mybir.ActivationFunctionType.Identity,
                                     bias=c_bnode[:])
                if debug and j0 == 0:
                    dt1_ = sb.tile([H, TN], dt.float32, name="dt1_", tag="dbg")
                    nc.vector.tensor_copy(dt1_[:, :w], t1[:, :w])
                    nc.sync.dma_start(out=dbg_t1[:, :w], in_=dt1_[:, :w])

                p2 = mps.tile([H, TN], dt.float32, tag="mps")
                nc.tensor.matmul(p2[:, :w], lhsT=c_Wlin[:], rhs=t1[:, :w],
                                 start=True, stop=True)
                t2 = sb.tile([H, TN], dt.bfloat16, tag="t1")
                act_silu(t2[:, :w], p2[:, :w], bias=c_blin[:])
                if debug and j0 == 0:
                    dt2_ = sb.tile([H, TN], dt.float32, name="dt2_", tag="dbg")
                    nc.vector.tensor_copy(dt2_[:, :w], t2[:, :w])
                    nc.sync.dma_start(out=dbg_t2[:, :w], in_=dt2_[:, :w])
                p3 = mps.tile([H, TN], dt.float32, tag="mps")
                nc.tensor.matmul(p3[:, :w], lhsT=c_Wlin2[:], rhs=t2[:, :w],
                                 start=True, stop=True)
                act_silu(hT[:, j0: j0 + w], p3[:, :w], bias=c_blin2[:])
                nc.vector.tensor_copy(hTb[:, j0: j0 + w], hT[:, j0: j0 + w])

            # ============ embedding: edge features eT ============
            for j0 in range(0, ES, TN):
                ww = min(TN, ES - j0)
                rt = sb.tile([3, TN], dt.bfloat16, tag="rt")
                nc.sync.dma_start(out=rt[:, :ww], in_=relT[:, j0: j0 + ww])
                at = sb.tile([G, TN], dt.bfloat16, tag="at")
                nc.sync.dma_start(out=at[:, :ww], in_=attrT[:, j0: j0 + ww])
                pe = mps.tile([F, TN], dt.float32, tag="mps")
                nc.tensor.matmul(pe[0:64, :ww], lhsT=c_We1[:], rhs=rt[:, :ww],
                                 start=True, stop=True)
                nc.tensor.matmul(pe[64:128, :ww], lhsT=c_We2[:], rhs=at[:, :ww],
                                 start=True, stop=True, tile_position=(0, 64))
                em = sb.tile([F, TN], dt.bfloat16, tag="t1")
                act_silu(em[:, :ww], pe[:, :ww], bias=c_be12[:])
                pf = mps.tile([F, TN], dt.float32, tag="mps")
                nc.tensor.matmul(pf[:, :ww], lhsT=c_We3[:], rhs=em[:, :ww],
                                 start=True, stop=True)
                et = sb.tile([F, TN], dt.bfloat16, tag="t1")
                act_silu(et[:, :ww], pf[:, :ww], bias=c_be3[:])
                nc.sync.dma_start(out=eT_dram[:, j0: j0 + ww], in_=et[:, :ww])

            if debug:
                for j0 in range(0, NSHARD, TN):
                    w = min(TN, NSHARD - j0)
                    dtt = sb.tile([H, TN], dt.float32, name=f"dt{j0}", tag="dbg")
                    nc.vector.tensor_copy(dtt[:, :w], hT[:, j0: j0 + w])
                    nc.sync.dma_start(out=dbg_h0[:, j0: j0 + w], in_=dtt[:, :w])
                for j0 in range(0, ES, TN):
                    ww = min(TN, ES - j0)
                    dte = sb.tile([P, TN], dt.float32, name=f"de{j0}", tag="dbg")
                    dts = sb.tile([P, TN], dt.bfloat16, name=f"ds{j0}", tag="dbg2")
                    nc.sync.dma_start(out=dts[:, :ww], in_=eT_dram[:, j0: j0 + ww])
                    nc.vector.tensor_copy(dte[:, :ww], dts[:, :ww])
                    nc.sync.dma_start(out=dbg_eT[:, j0: j0 + ww], in_=dte[:, :ww])

            # ============ layers ============
            for l in range(L):
                # ---- node phase: hd = silu(h @ Wdown + bdown) -> allgather
                for w0 in range(0, NW, 4):
                    nwin = min(4, NW - w0)
                    hdt = sb.tile([P, 4, F], dt.bfloat16, tag="hd4")
                    for a in range(nwin):
                        w = w0 + a
                        php = sps.tile([P, F], dt.float32, tag="sps")
                        nc.tensor.matmul(php[:], lhsT=c_ones1[:],
                                         rhs=c_bdown1[:, l, :], start=True, stop=False)
                        nc.tensor.matmul(php[:], lhsT=hTb[:, w * P:(w + 1) * P],
                                         rhs=c_Wdown[:, l, :], start=False, stop=True)
                        act_silu(hdt[:, a, :], php[:])
                    nc.sync.dma_start(
                        out=hd_local[:].rearrange("(a p) d -> p a d", p=P)[
                            :, w0: w0 + nwin, :],
                        in_=hdt[:, :nwin, :])
                nc.gpsimd.collective_compute(
                    "AllGather", AL.bypass,
                    replica_groups=[list(range(NC_))],
                    ins=[hd_local[:].opt()], outs=[hd_full[:].opt()])

                # ---- edge phase ----
                agg_open = {}
                for call_i, (base_ch, ncall, half) in enumerate(calls):
                    gt = gat.tile([P, GCALL, F], dt.bfloat16, tag="hdg")
                    ni = ncall * P
                    nc.gpsimd.dma_gather(
                        gt[:, :ncall, :],
                        hd_full[half * (NC_ // 2) * NSHARD:, :],
                        c_idx[:, base_ch * 8: base_ch * 8 + ni // 16],
                        ni, ni, F, queue_num=call_i % 4)
                    eTt = wpool.tile([P, GCALL, F], dt.bfloat16, tag="eTt", bufs=6)
                    nc.sync.dma_start(
                        out=eTt[:, :ncall, :],
                        in_=eT_dram[:, base_ch * P: base_ch * P + ni].rearrange(
                            "p (c q) -> p c q", q=P))
                    for s0 in range(0, ncall, WSUB):
                        nsub = min(WSUB, ncall - s0)
                        wp = mps.tile([P, WSUB, F], dt.float32, tag="mps")
                        nc.tensor.matmul(
                            wp[:, :nsub, :].rearrange("p a q -> p (a q)"),
                            lhsT=c_ones1[:],
                            rhs=c_bgeom8[:, l, : nsub * F],
                            start=True, stop=False)
                        for ci in range(nsub):
                            nc.tensor.matmul(
                                wp[:, ci, :], lhsT=eTt[:, s0 + ci, :],
                                rhs=c_Wgeom[:, l, :], start=False, stop=(ci == nsub - 1))
                        wsb = wpool.tile([P, WSUB, F], dt.bfloat16, tag="wsb", bufs=16)
                        act_silu(wsb[:, :nsub, :], wp[:, :nsub, :])
                        msg = sb.tile([P, WSUB, F], dt.bfloat16, tag="msg")
                        nc.vector.tensor_mul(msg[:, :nsub, :], wsb[:, :nsub, :],
                                             gt[:, s0: s0 + nsub, :])
                        oh = sb.tile([P, WSUB, F], dt.bfloat16, tag="oh")
                        nc.vector.tensor_tensor(
                            out=oh[:, :nsub, :],
                            in0=c_iota8[:, :nsub, :],
                            in1=c_dstloc[:, base_ch + s0: base_ch + s0 + nsub
                                         ].to_broadcast([P, nsub, P]),
                            op=AL.is_equal)
                        for ci in range(nsub):
                            ch = base_ch + s0 + ci
                            w = int(chunk_win[ch])
                            in_lo = ch < NLOC
                            q = QLO if in_lo else QHI
                            rel_c = ch - (0 if in_lo else NLOC)
                            first = rel_c % q == 0
                            last = rel_c % q == q - 1
                            key = (w, in_lo)
                            if first:
                                agg_open[key] = aggps.tile(
                                    [P, F], dt.float32, tag="aggps",
                                    name=f"aggp_{l}_{ch}")
                            nc.tensor.matmul(agg_open[key][:], lhsT=oh[:, ci, :],
                                             rhs=msg[:, ci, :],
                                             start=first, stop=last)
                            if last:
                                if in_lo:
                                    nc.vector.tensor_copy(agg_sb[:, w, :],
                                                          agg_open[key][:])
                                else:
                                    nc.vector.tensor_add(agg_sb[:, w, :],
                                                         agg_sb[:, w, :],
                                                         agg_open[key][:])
                                del agg_open[key]

                if debug and l == 0:
                    for w in range(NW):
                        dta = sb.tile([P, F], dt.float32, name=f"da{w}", tag="dbg")
                        nc.vector.tensor_copy(dta[:], agg_sb[:, w, :])
                        nc.sync.dma_start(
                            out=dbg_agg[:, w, :], in_=dta[:])
                    for w in range(NW):
                        dhs = sb.tile([P, F], dt.bfloat16, name=f"dq{w}", tag="dbg2")
                        nc.sync.dma_start(
                            out=dhs[:],
                            in_=hd_local[:].rearrange("(a p) d -> p a d", p=P)[:, w, :])
                        dth = sb.tile([P, F], dt.float32, name=f"dh{w}", tag="dbg")
                        nc.vector.tensor_copy(dth[:], dhs[:])
                        nc.sync.dma_start(
                            out=dbg_hd.rearrange("(a p) d -> p a d", p=P)[:, w, :],
                            in_=dth[:])

                # ---- GraphNorm + update ----
                gsum_p = gps.tile([GPC, H], dt.float32, tag="gps")
                for w in range(NW):
                    nc.tensor.matmul(gsum_p[:], lhsT=c_boh[:, w, :],
                                     rhs=agg_sb[:, w, :],
                                     start=(w == 0), stop=(w == NW - 1))
                tmean = sb2.tile([GPC, H], dt.float32, tag="gn32")
                nc.vector.tensor_scalar(out=tmean[:], in0=gsum_p[:],
                                        scalar1=c_cnt_inv[:], scalar2=None,
                                        op0=AL.mult)
                mean_sc = sb2.tile([GPC, H], dt.bfloat16, tag="gn")
                nc.vector.tensor_mul(mean_sc[:], tmean[:], c_gnmsB[:, l, :])
                for w in range(NW):
                    mb = sps.tile([P, H], dt.float32, tag="sps")
                    nc.tensor.matmul(mb[:], lhsT=c_bohT[:, w, :], rhs=mean_sc[:],
                                     start=True, stop=True)
                    nc.vector.tensor_sub(ctr_sb[:, w, :], agg_sb[:, w, :], mb[:])
                sq_p = gps.tile([GPC, H], dt.float32, tag="gps")
                for w in range(NW):
                    sq = sb2.tile([P, H], dt.bfloat16, tag="sq")
                    nc.vector.tensor_mul(sq[:], ctr_sb[:, w, :], ctr_sb[:, w, :])
                    nc.tensor.matmul(sq_p[:], lhsT=c_boh[:, w, :], rhs=sq[:],
                                     start=(w == 0), stop=(w == NW - 1))
                var = sb2.tile([GPC, H], dt.float32, tag="gn32")
                nc.vector.tensor_scalar(out=var[:], in0=sq_p[:],
                                        scalar1=c_cnt_inv[:], scalar2=None,
                                        op0=AL.mult)
                sd = sb2.tile([GPC, H], dt.float32, tag="gn32")
                nc.scalar.activation(sd[:], var[:], SQT, bias=c_eps[:])
                rs = sb2.tile([GPC, H], dt.float32, tag="gn32")
                nc.vector.reciprocal(rs[:], sd[:])
                scale = sb2.tile([GPC, H], dt.bfloat16, tag="gn")
                nc.vector.tensor_mul(scale[:], rs[:], c_gnwB[:, l, :])
                for w0 in range(0, NW, 4):
                    nwin = min(4, NW - w0)
                    hnT4 = sb2.tile([F, 4 * P], dt.bfloat16, tag="hnT4")
                    for a in range(nwin):
                        w = w0 + a
                        sbp = sps.tile([P, H], dt.float32, tag="sps")
                        nc.tensor.matmul(sbp[:], lhsT=c_bohT[:, w, :], rhs=scale[:],
                                         start=True, stop=True)
                        hn = sb2.tile([P, H], dt.float32, tag="hn")
                        nc.vector.tensor_mul(hn[:], ctr_sb[:, w, :], sbp[:])
                        hn2 = sb2.tile([P, H], dt.float32, tag="hn2")
                        nc.vector.tensor_add(hn2[:], hn[:], c_gnbB[:, l, :])
                        shn = sb2.tile([P, H], dt.bfloat16, tag="shn")
                        act_silu(shn[:], hn2[:])
                        tp = sps.tile([P, P], dt.bfloat16, tag="sps")
                        nc.tensor.transpose(tp[:], shn[:], c_ident[:])
                        nc.vector.tensor_copy(hnT4[:, a * P:(a + 1) * P], tp[:])
                    upp = mps.tile([H, 4 * P], dt.float32, tag="mps")
                    nc.tensor.matmul(upp[:, : nwin * P], lhsT=c_Wup[:, l, :],
                                     rhs=hnT4[:, : nwin * P], start=True, stop=True)
                    ups = sb2.tile([H, 4 * P], dt.float32, tag="ups")
                    act_silu(ups[:, : nwin * P], upp[:, : nwin * P], bias=c_bup[:, l: l + 1])
                    nc.vector.tensor_add(hT[:, w0 * P: w0 * P + nwin * P],
                                         hT[:, w0 * P: w0 * P + nwin * P],
                                         ups[:, : nwin * P])
                    nc.vector.tensor_copy(hTb[:, w0 * P: w0 * P + nwin * P],
                                          hT[:, w0 * P: w0 * P + nwin * P])

                if debug and l == 0:
                    for j0 in range(0, NSHARD, TN):
                        w_ = min(TN, NSHARD - j0)
                        dt1 = sb.tile([H, TN], dt.float32, name=f"d1{j0}", tag="dbg")
                        nc.vector.tensor_copy(dt1[:, :w_], hT[:, j0: j0 + w_])
                        nc.sync.dma_start(out=dbg_h1[:, j0: j0 + w_], in_=dt1[:, :w_])

            # ============ output block ============
            z_p = gps.tile([GPC, 64], dt.float32, tag="gps")
            for w in range(NW):
                t3p = sps.tile([P, 64], dt.float32, tag="sps")
                nc.tensor.matmul(t3p[:], lhsT=c_ones1[:], rhs=c_bo11[:],
                                 start=True, stop=False)
                nc.tensor.matmul(t3p[:], lhsT=hTb[:, w * P:(w + 1) * P],
                                 rhs=c_Wo1[:], start=False, stop=True)
                t3 = sb2.tile([P, 64], dt.bfloat16, tag="t3b")
                act_silu(t3[:], t3p[:])
                nc.tensor.matmul(z_p[:], lhsT=c_boh[:, w, :], rhs=t3[:],
                                 start=(w == 0), stop=(w == NW - 1))
            z_sb = sb2.tile([GPC, 64], dt.bfloat16, tag="zsb")
            nc.vector.tensor_copy(z_sb[:], z_p[:])
            zT_p = sps.tile([64, GPC], dt.bfloat16, tag="sps")
            nc.tensor.transpose(zT_p[:], z_sb[:], c_ident[:GPC, :GPC])
            zT = sb2.tile([64, GPC], dt.bfloat16, tag="zT")
            nc.vector.tensor_copy(zT[:], zT_p[:])
            en_p = sps.tile([GPC, 1], dt.float32, tag="sps")
            nc.tensor.matmul(en_p[:], lhsT=zT[:], rhs=c_Wo2[:],
                             start=True, stop=True)
            en = sb2.tile([GPC, 1], dt.float32, tag="en")
            nc.vector.tensor_add(en[:], en_p[:], c_cntbo2[:])
            nc.sync.dma_start(out=energy[:], in_=en[:])

    nc.compile()
    return nc


def _install_ntff_hook():
    """Restore antenv.axon_hooks + register the ctypes NTFF hook."""
    import types

    try:
        from antenv.axon_hooks import get_axon_ntff_profile_hook  # noqa: F401

        return
    except ImportError:
        pass
    try:
        import antenv

        mod = types.ModuleType("antenv.axon_hooks")
        mod._hook = None

        def _set(h):
            mod._hook = h

        def _get():
            return mod._hook

        mod.set_axon_ntff_profile_hook = _set
        mod.get_axon_ntff_profile_hook = _get
        sys.modules["antenv.axon_hooks"] = mod
        antenv.axon_hooks = mod
        sys.path.insert(0, "/root/.axon_site")
        from trn_agent_boot.trn_boot import _ntff_profile_via_ctypes

        hook = _ntff_profile_via_ctypes("/opt/axon/libaxon_pjrt.so")
        if hook is not None:
            _set(hook)
    except Exception as e:  # pragma: no cover
        print(f"ntff hook install failed: {e}", file=sys.stderr)


def kernel(**inputs) -> np.ndarray:
    meta, in_maps, _extra = _build_host(inputs)
    nc = _build_program(meta)
    from concourse.bass_utils import run_bass_kernel_spmd

    trace = bool(int(os.environ.get("KERNEL_TRACE", "0")))
    if trace:
        _install_ntff_hook()
    res = run_bass_kernel_spmd(nc, in_maps, core_ids=list(range(NC_)), trace=trace)
    if trace:
        kernel.last_results = res
    out = np.concatenate([res.results[k]["energy"] for k in range(NC_)], axis=0)
    return out.astype(np.float32)



# revision 20
# speedup vs baseline: 1.1389x; 1.0000x over previous
"""FAENet-style GNN message passing on 8 Trainium2 NeuronCores (Bass/Tile).

Sharding: nodes by graph id (contiguous since `batch` is sorted) -> 8 graphs
per core; edges assigned to the core owning their dst node. Per-layer
cross-core AllGather of the down-projected node features (hd) feeds the
src-side gathers. Scatter-add (segment_sum over dst) runs on the tensor
engine as onehot^T @ msg per 128-node window; src gathers use the SWDGE
dma_gather instruction (int16 indices -> hd table split in two halves).
"""

import os
import sys

import numpy as np

for _p in ("/opt/trn_rl_repo", "/root/.axon_site/_ro/trn_rl_repo"):
    if _p not in sys.path and os.path.isdir(_p):
        sys.path.insert(0, _p)

import ml_dtypes  # noqa: E402

BF16 = ml_dtypes.bfloat16

N, E, H, F, G, C, L = 50000, 800000, 128, 128, 50, 92, 4
N_GRAPHS = 64
CUTOFF = 6.0
EPS = 1e-5
NC_ = 8  # cores
P = 128
GPC = N_GRAPHS // NC_  # graphs per core
GCALL = 8  # max chunks per dma_gather call (1024 idx)
WSUB = 4  # chunks per W-psum group


def _build_host(inputs):
    """All integer/index preprocessing + constant tables, per core."""
    x = np.asarray(inputs["x"], np.float32)
    pos = np.asarray(inputs["pos"], np.float32)
    ei = np.asarray(inputs["edge_index"]).astype(np.int64)
    batch = np.asarray(inputs["batch"]).astype(np.int64)
    src, dst = ei[0], ei[1]

    gstart = np.searchsorted(batch, np.arange(0, N_GRAPHS + 1, GPC))
    ns, ne = gstart[:-1], gstart[1:]
    nk = ne - ns
    NSHARD = int(((nk.max() + P - 1) // P) * P)
    NW = NSHARD // P
    HALF = (NC_ // 2) * NSHARD
    assert HALF < 32768

    core_of_node = np.repeat(np.arange(NC_), nk)
    trow = (core_of_node * NSHARD + (np.arange(N) - ns[core_of_node])).astype(np.int64)
    edge_core = core_of_node[dst]

    per_core = []
    qlo_max, qhi_max = 1, 1
    for k in range(NC_):
        em = np.nonzero(edge_core == k)[0]
        s_k, d_k = src[em], dst[em]
        dloc = d_k - ns[k]
        win = dloc // P
        srow = trow[s_k]
        lo = srow < HALF
        order = np.lexsort((~lo, win))
        em, dloc, win, srow, lo = (
            em[order], dloc[order], win[order], srow[order], lo[order])
        nlo = np.bincount(win[lo], minlength=NW)
        nhi = np.bincount(win[~lo], minlength=NW)
        per_core.append((em, dloc, win, srow, lo, nlo, nhi))
        qlo_max = max(qlo_max, int(np.ceil(nlo.max() / P)))
        qhi_max = max(qhi_max, int(np.ceil(nhi.max() / P)))

    QLO, QHI = qlo_max, qhi_max
    NLOC = NW * QLO
    NCHUNK = NW * (QLO + QHI)
    ES = NCHUNK * P

    rel = pos[src] - pos[dst]
    distf = np.sqrt((rel * rel).sum(1) + 1e-12)
    off = np.linspace(0.0, CUTOFF, G).astype(np.float32)
    coeff = -0.5 / (off[1] - off[0]) ** 2
    attr_all = np.exp(coeff * (distf[:, None] - off[None, :]) ** 2).astype(np.float32)

    def call_plan(nchunks):
        calls, c = [], 0
        while c < nchunks:
            n = min(GCALL, nchunks - c)
            calls.append((c, n))
            c += n
        return calls

    calls_lo = call_plan(NW * QLO)
    calls_hi = call_plan(NW * QHI)
    chunk_win = np.concatenate(
        [np.repeat(np.arange(NW), QLO), np.repeat(np.arange(NW), QHI)])

    meta = dict(NSHARD=NSHARD, NW=NW, HALF=HALF, QLO=QLO, QHI=QHI,
                NCHUNK=NCHUNK, ES=ES, NLOC=NLOC,
                calls_lo=calls_lo, calls_hi=calls_hi, chunk_win=chunk_win)

    in_maps = []
    for k in range(NC_):
        em, dloc, win, srow, lo, nlo, nhi = per_core[k]
        slot = np.full(ES, -1, np.int64)
        sdst = np.full(ES, 255, np.int64)
        stab = np.zeros(ES, np.int64)
        pos_lo, pos_hi = np.nonzero(lo)[0], np.nonzero(~lo)[0]
        ofs_lo = np.concatenate(([0], np.cumsum(nlo)))
        ofs_hi = np.concatenate(([0], np.cumsum(nhi)))
        for w in range(NW):
            a, b = int(ofs_lo[w]), int(ofs_lo[w + 1])
            sl0 = w * QLO * P
            idxs = pos_lo[a:b]
            slot[sl0: sl0 + b - a] = em[idxs]
            sdst[sl0: sl0 + b - a] = dloc[idxs] % P
            stab[sl0: sl0 + b - a] = srow[idxs]
            a, b = int(ofs_hi[w]), int(ofs_hi[w + 1])
            sl0 = (NLOC + w * QHI) * P
            idxs = pos_hi[a:b]
            slot[sl0: sl0 + b - a] = em[idxs]
            sdst[sl0: sl0 + b - a] = dloc[idxs] % P
            stab[sl0: sl0 + b - a] = srow[idxs] - HALF
        valid = slot >= 0
        eids = np.where(valid, slot, 0)

        relT = np.where(valid[None, :], rel[eids].T, 0.0).astype(BF16)
        attrT = np.where(valid[None, :], attr_all[eids].T, 0.0).astype(BF16)

        sidx = np.where(valid, stab, 0).astype(np.int64)
        blocks = []
        for base_ch, ncall in calls_lo + [(NLOC + c, n) for c, n in calls_hi]:
            ni = ncall * P
            vv = sidx[base_ch * P: base_ch * P + ni]
            blk = vv.reshape(ni // 16, 16).T.astype(np.int16)
            blocks.append(np.tile(blk, (8, 1)))
        idxcat = np.ascontiguousarray(np.concatenate(blocks, axis=1))

        dstloc = np.ascontiguousarray(
            sdst.reshape(NCHUNK, P).T.astype(np.float32)).astype(BF16)

        bloc = np.full(NSHARD, GPC, np.int64)
        bloc[: nk[k]] = batch[ns[k]: ne[k]] - k * GPC
        boh = np.zeros((NSHARD, GPC), np.float32)
        m = bloc < GPC
        boh[np.nonzero(m)[0], bloc[m]] = 1.0
        bonehot = np.ascontiguousarray(
            boh.reshape(NW, P, GPC).transpose(1, 0, 2)).astype(BF16)
        bonehotT = np.ascontiguousarray(
            boh.reshape(NW, P, GPC).transpose(2, 0, 1)).astype(BF16)
        cnt = np.maximum(np.bincount(bloc[m], minlength=GPC), 1.0).astype(np.float32)
        cnt_inv = np.ascontiguousarray((1.0 / cnt).reshape(GPC, 1))
        cntbo2 = (np.bincount(bloc[m], minlength=GPC).astype(np.float32)
                  * float(np.asarray(inputs["bo2"]).reshape(-1)[0])).reshape(GPC, 1)

        xT = np.zeros((C, NSHARD), np.float32)
        xT[:, : nk[k]] = x[ns[k]: ne[k]].T
        xT = xT.astype(BF16)

        in_maps.append(dict(relT=relT, attrT=attrT, idxcat=idxcat, dstloc=dstloc,
                            bonehot=bonehot, bonehotT=bonehotT, cnt_inv=cnt_inv,
                            cntbo2=np.ascontiguousarray(cntbo2), xT=xT))

    w32 = lambda a: np.ascontiguousarray(np.asarray(a, np.float32))
    wbf = lambda a: np.ascontiguousarray(np.asarray(a, np.float32)).astype(BF16)
    iota = np.arange(P, dtype=np.float32)
    shared = dict(
        We1=wbf(inputs["We1"]),
        We2=wbf(inputs["We2"]),
        We3=wbf(inputs["We3"]),
        be12=w32(np.concatenate([np.asarray(inputs["be1"]),
                                 np.asarray(inputs["be2"])])).reshape(F, 1),
        be3=w32(inputs["be3"]).reshape(F, 1),
        Wnode=wbf(inputs["Wnode"]),
        Wlin=wbf(inputs["Wlin"]),
        Wlin2=wbf(inputs["Wlin2"]),
        bnode=w32(inputs["bnode"]).reshape(H, 1),
        blin=w32(inputs["blin"]).reshape(H, 1),
        blin2=w32(inputs["blin2"]).reshape(H, 1),
        Wgeom=wbf(np.transpose(np.asarray(inputs["Wgeom"], np.float32), (1, 0, 2))),
        Wdown=wbf(np.transpose(np.asarray(inputs["Wdown"], np.float32), (1, 0, 2))),
        Wup=wbf(np.transpose(np.asarray(inputs["Wup"], np.float32), (1, 0, 2))),
        bgeom8=wbf(np.tile(np.asarray(inputs["bgeom"], np.float32),
                           (1, GCALL))[None, :, :]),
        bdown1=wbf(np.asarray(inputs["bdown"], np.float32)[None, :, :]),
        bup=w32(np.asarray(inputs["bup"], np.float32).T),
        gnmsB=w32(np.tile(np.asarray(inputs["gnms"], np.float32)[None, :, :],
                          (GPC, 1, 1))),
        gnwB=w32(np.tile(np.asarray(inputs["gnw"], np.float32)[None, :, :],
                         (GPC, 1, 1))),
        gnbB=w32(np.tile(np.asarray(inputs["gnb"], np.float32)[None, :, :],
                         (P, 1, 1))),
        Wo1=wbf(inputs["Wo1"]),
        bo11=wbf(np.asarray(inputs["bo1"], np.float32)[None, :]),
        Wo2=wbf(inputs["Wo2"]),
        ones1=np.ones((1, P), np.float32).astype(BF16),
        iota8=np.ascontiguousarray(
            np.tile(iota[None, None, :], (P, GCALL, 1))).astype(BF16),
        identity=np.eye(P, dtype=np.float32).astype(BF16),
    )
    for m_ in in_maps:
        m_.update(shared)
    return meta, in_maps, dict(ns=ns, ne=ne, nk=nk)


def _build_program(meta):
    import concourse.bass as bass  # noqa: F401
    import concourse.tile as tile
    from concourse import bacc, library_config, mybir

    dt = mybir.dt
    NSHARD, NW = meta["NSHARD"], meta["NW"]
    NCHUNK, ES, NLOC = meta["NCHUNK"], meta["ES"], meta["NLOC"]
    QLO, QHI = meta["QLO"], meta["QHI"]
    chunk_win = meta["chunk_win"]
    calls = [(c, n, 0) for c, n in meta["calls_lo"]] + [
        (NLOC + c, n, 1) for c, n in meta["calls_hi"]]

    nc = bacc.Bacc("TRN2", target_bir_lowering=False, num_devices=NC_,
                   num_swdge_queues=4)

    def din(name, shape, d=dt.float32):
        return nc.dram_tensor(name, shape, d, kind="ExternalInput")

    relT = din("relT", [3, ES], dt.bfloat16)
    attrT = din("attrT", [G, ES], dt.bfloat16)
    idxcat = din("idxcat", [P, ES // 16], dt.int16)
    dstloc = din("dstloc", [P, NCHUNK], dt.bfloat16)
    bonehot = din("bonehot", [P, NW, GPC], dt.bfloat16)
    bonehotT = din("bonehotT", [GPC, NW, P], dt.bfloat16)
    cnt_inv = din("cnt_inv", [GPC, 1])
    cntbo2 = din("cntbo2", [GPC, 1])
    xT = din("xT", [C, NSHARD], dt.bfloat16)
    We1 = din("We1", [3, 64], dt.bfloat16)
    We2 = din("We2", [G, 64], dt.bfloat16)
    We3 = din("We3", [F, F], dt.bfloat16)
    be12 = din("be12", [F, 1])
    be3 = din("be3", [F, 1])
    Wnode = din("Wnode", [C, H], dt.bfloat16)
    Wlin = din("Wlin", [H, H], dt.bfloat16)
    Wlin2 = din("Wlin2", [H, H], dt.bfloat16)
    bnode = din("bnode", [H, 1])
    blin = din("blin", [H, 1])
    blin2 = din("blin2", [H, 1])
    Wgeom = din("Wgeom", [F, L, F], dt.bfloat16)
    Wdown = din("Wdown", [H, L, F], dt.bfloat16)
    Wup = din("Wup", [F, L, H], dt.bfloat16)
    bgeom8 = din("bgeom8", [1, L, GCALL * F], dt.bfloat16)
    bdown1 = din("bdown1", [1, L, F], dt.bfloat16)
    bup = din("bup", [H, L])
    gnmsB = din("gnmsB", [GPC, L, H])
    gnwB = din("gnwB", [GPC, L, H])
    gnbB = din("gnbB", [P, L, H])
    Wo1 = din("Wo1", [H, 64], dt.bfloat16)
    bo11 = din("bo11", [1, 64], dt.bfloat16)
    Wo2 = din("Wo2", [64, 1], dt.bfloat16)
    ones1 = din("ones1", [1, P], dt.bfloat16)
    iota8 = din("iota8", [P, GCALL, P], dt.bfloat16)
    identity = din("identity", [P, P], dt.bfloat16)

    energy = nc.dram_tensor("energy", [GPC, 1], dt.float32, kind="ExternalOutput")
    debug = bool(int(os.environ.get("KERNEL_DEBUG", "0")))
    if debug:
        dbg_h0 = nc.dram_tensor("dbg_h0", [H, NSHARD], dt.float32, kind="ExternalOutput")
        dbg_eT = nc.dram_tensor("dbg_eT", [P, ES], dt.float32, kind="ExternalOutput")
        dbg_agg = nc.dram_tensor("dbg_agg", [P, NW, F], dt.float32, kind="ExternalOutput")
        dbg_h1 = nc.dram_tensor("dbg_h1", [H, NSHARD], dt.float32, kind="ExternalOutput")
        dbg_hd = nc.dram_tensor("dbg_hd", [NSHARD, H], dt.float32, kind="ExternalOutput")
        dbg_p1 = nc.dram_tensor("dbg_p1", [H, 512], dt.float32, kind="ExternalOutput")
        dbg_t2 = nc.dram_tensor("dbg_t2", [H, 512], dt.float32, kind="ExternalOutput")
        dbg_t1 = nc.dram_tensor("dbg_t1", [H, 512], dt.float32, kind="ExternalOutput")

    SI = mybir.ActivationFunctionType.Silu
    SQT = mybir.ActivationFunctionType.Sqrt
    AL = mybir.AluOpType

    with tile.TileContext(nc) as tc:
        with (
            tc.tile_pool(name="dram", bufs=1, space="DRAM") as dram,
            tc.tile_pool(name="const", bufs=1) as cpool,
            tc.tile_pool(name="big", bufs=1) as bigp,
            tc.tile_pool(name="sb", bufs=3) as sb,
            tc.tile_pool(name="wpool", bufs=8) as wpool,
            tc.tile_pool(name="sb2", bufs=3) as sb2,
            tc.tile_pool(name="gat", bufs=12) as gat,
            tc.tile_pool(name="mps", bufs=3, space="PSUM") as mps,
            tc.tile_pool(name="aggps", bufs=2, space="PSUM") as aggps,
            tc.tile_pool(name="sps", bufs=2, space="PSUM") as sps,
            tc.tile_pool(name="gps", bufs=1, space="PSUM") as gps,
        ):
            with tc.tile_critical():
                nc.gpsimd.load_library(library_config.mlp)

            hd_local = dram.tile([NSHARD, H], dt.bfloat16)
            hd_full = nc.dram_tensor(
                "hd_full_sh", [NC_ * NSHARD, H], dt.bfloat16,
                kind="Internal", addr_space="Shared")
            eT_dram = dram.tile([P, ES], dt.bfloat16)

            _cn = [0]

            def cload(src, shape, d=dt.float32):
                _cn[0] += 1
                t = cpool.tile(shape, d, name=f"cst{_cn[0]}", tag=f"cst{_cn[0]}")
                nc.sync.dma_start(out=t[:], in_=src)
                return t

            c_We1 = cload(We1[:], [3, 64], dt.bfloat16)
            c_We2 = cload(We2[:], [G, 64], dt.bfloat16)
            c_We3 = cload(We3[:], [F, F], dt.bfloat16)
            c_be12 = cload(be12[:], [F, 1])
            c_be3 = cload(be3[:], [F, 1])
            c_Wnode = cload(Wnode[:], [C, H], dt.bfloat16)
            c_Wlin = cload(Wlin[:], [H, H], dt.bfloat16)
            c_Wlin2 = cload(Wlin2[:], [H, H], dt.bfloat16)
            c_bnode = cload(bnode[:], [H, 1])
            c_blin = cload(blin[:], [H, 1])
            c_blin2 = cload(blin2[:], [H, 1])
            c_Wgeom = cload(Wgeom[:], [F, L, F], dt.bfloat16)
            c_Wdown = cload(Wdown[:], [H, L, F], dt.bfloat16)
            c_Wup = cload(Wup[:], [F, L, H], dt.bfloat16)
            c_bgeom8 = cload(bgeom8[:], [1, L, GCALL * F], dt.bfloat16)
            c_bdown1 = cload(bdown1[:], [1, L, F], dt.bfloat16)
            c_bup = cload(bup[:], [H, L])
            c_gnmsB = cload(gnmsB[:], [GPC, L, H])
            c_gnwB = cload(gnwB[:], [GPC, L, H])
            c_gnbB = cload(gnbB[:], [P, L, H])
            c_Wo1 = cload(Wo1[:], [H, 64], dt.bfloat16)
            c_bo11 = cload(bo11[:], [1, 64], dt.bfloat16)
            c_Wo2 = cload(Wo2[:], [64, 1], dt.bfloat16)
            c_ones1 = cload(ones1[:], [1, P], dt.bfloat16)
            c_iota8 = cload(iota8[:], [P, GCALL, P], dt.bfloat16)
            c_ident = cload(identity[:], [P, P], dt.bfloat16)
            c_cnt_inv = cload(cnt_inv[:], [GPC, 1])
            c_cntbo2 = cload(cntbo2[:], [GPC, 1])
            c_boh = cload(bonehot[:], [P, NW, GPC], dt.bfloat16)
            c_bohT = cload(bonehotT[:], [GPC, NW, P], dt.bfloat16)
            c_dstloc = cload(dstloc[:], [P, NCHUNK], dt.bfloat16)
            c_idx = cload(idxcat[:], [P, ES // 16], dt.int16)

            c_eps = cpool.tile([GPC, 1], dt.float32)
            nc.vector.memset(c_eps[:], EPS)

            sim_silu = bool(int(os.environ.get("KERNEL_SIM_SILU", "0")))
            silu_n = [0]

            def act_silu(out_ap, in_ap, bias=None):
                if not sim_silu:
                    if bias is None:
                        nc.scalar.activation(out_ap, in_ap, SI)
                    else:
                        nc.scalar.activation(out_ap, in_ap, SI, bias=bias)
                    return
                silu_n[0] += 1
                shp = list(in_ap.shape)
                pre = sb.tile(shp, dt.float32, name=f"slp{silu_n[0]}", tag="slp")
                ID = mybir.ActivationFunctionType.Identity
                SG = mybir.ActivationFunctionType.Sigmoid
                if bias is None:
                    nc.scalar.activation(pre[:], in_ap, ID)
                else:
                    nc.scalar.activation(pre[:], in_ap, ID, bias=bias)
                sg = sb.tile(shp, dt.float32, name=f"slg{silu_n[0]}", tag="slg")
                nc.scalar.activation(sg[:], pre[:], SG)
                nc.vector.tensor_mul(out_ap, pre[:], sg[:])

            hT = bigp.tile([H, NSHARD], dt.float32)
            hTb = bigp.tile([H, NSHARD], dt.bfloat16)
            agg_sb = bigp.tile([P, NW, F], dt.bfloat16)
            ctr_sb = bigp.tile([P, NW, F], dt.bfloat16)

            # ============ embedding: h0 = MLP(x) ============
            TN = 512
            for j0 in range(0, NSHARD, TN):
                w = min(TN, NSHARD - j0)
                xt = sb.tile([C, TN], dt.bfloat16, tag="xt")
                nc.sync.dma_start(out=xt[:, :w], in_=xT[:, j0: j0 + w])
                p1 = mps.tile([H, TN], dt.float32, tag="mps")
                nc.tensor.matmul(p1[:, :w], lhsT=c_Wnode[:], rhs=xt[:, :w],
                                 start=True, stop=True)
                t1 = sb.tile([H, TN], dt.bfloat16, tag="t1")
                nc.scalar.activation(t1[:, :w], p1[:, :w],
                                     mybir.ActivationFunctionType.Identity,
                                     bias=c_bnode[:])
                if debug and j0 == 0:
                    dt1_ = sb.tile([H, TN], dt.float32, name="dt1_", tag="dbg")
                    nc.vector.tensor_copy(dt1_[:, :w], t1[:, :w])
                    nc.sync.dma_start(out=dbg_t1[:, :w], in_=dt1_[:, :w])

                p2 = mps.tile([H, TN], dt.float32, tag="mps")
                nc.tensor.matmul(p2[:, :w], lhsT=c_Wlin[:], rhs=t1[:, :w],
                                 start=True, stop=True)
                t2 = sb.tile([H, TN], dt.bfloat16, tag="t1")
                act_silu(t2[:, :w], p2[:, :w], bias=c_blin[:])
                if debug and j0 == 0:
                    dt2_ = sb.tile([H, TN], dt.float32, name="dt2_", tag="dbg")
                    nc.vector.tensor_copy(dt2_[:, :w], t2[:, :w])
                    nc.sync.dma_start(out=dbg_t2[:, :w], in_=dt2_[:, :w])
                p3 = mps.tile([H, TN], dt.float32, tag="mps")
                nc.tensor.matmul(p3[:, :w], lhsT=c_Wlin2[:], rhs=t2[:, :w],
                                 start=True, stop=True)
                act_silu(hT[:, j0: j0 + w], p3[:, :w], bias=c_blin2[:])
                nc.vector.tensor_copy(hTb[:, j0: j0 + w], hT[:, j0: j0 + w])

            # ============ embedding: edge features eT (emitted JIT) ============
            def emit_embed(j0):
                ww = min(TN, ES - j0)
                rt = sb.tile([3, TN], dt.bfloat16, tag="rt", name=f"rt{j0}")
                nc.sync.dma_start(out=rt[:, :ww], in_=relT[:, j0: j0 + ww])
                at = sb.tile([G, TN], dt.bfloat16, tag="at", name=f"at{j0}")
                nc.sync.dma_start(out=at[:, :ww], in_=attrT[:, j0: j0 + ww])
                pe = mps.tile([F, TN], dt.float32, tag="mps", name=f"pe{j0}")
                nc.tensor.matmul(pe[0:64, :ww], lhsT=c_We1[:], rhs=rt[:, :ww],
                                 start=True, stop=True)
                nc.tensor.matmul(pe[64:128, :ww], lhsT=c_We2[:], rhs=at[:, :ww],
                                 start=True, stop=True, tile_position=(0, 64))
                em = sb.tile([F, TN], dt.bfloat16, tag="t1", name=f"em{j0}")
                act_silu(em[:, :ww], pe[:, :ww], bias=c_be12[:])
                pf = mps.tile([F, TN], dt.float32, tag="mps", name=f"pf{j0}")
                nc.tensor.matmul(pf[:, :ww], lhsT=c_We3[:], rhs=em[:, :ww],
                                 start=True, stop=True)
                et = sb.tile([F, TN], dt.bfloat16, tag="t1", name=f"et{j0}")
                act_silu(et[:, :ww], pf[:, :ww], bias=c_be3[:])
                nc.sync.dma_start(out=eT_dram[:, j0: j0 + ww], in_=et[:, :ww])

            embed_next = [0]

            def ensure_embed(cols_needed):
                while embed_next[0] < min(cols_needed, ES):
                    emit_embed(embed_next[0])
                    embed_next[0] += TN

            if debug:
                for j0 in range(0, NSHARD, TN):
                    w = min(TN, NSHARD - j0)
                    dtt = sb.tile([H, TN], dt.float32, name=f"dt{j0}", tag="dbg")
                    nc.vector.tensor_copy(dtt[:, :w], hT[:, j0: j0 + w])
                    nc.sync.dma_start(out=dbg_h0[:, j0: j0 + w], in_=dtt[:, :w])
                for j0 in range(0, ES, TN):
                    ww = min(TN, ES - j0)
                    dte = sb.tile([P, TN], dt.float32, name=f"de{j0}", tag="dbg")
                    dts = sb.tile([P, TN], dt.bfloat16, name=f"ds{j0}", tag="dbg2")
                    nc.sync.dma_start(out=dts[:, :ww], in_=eT_dram[:, j0: j0 + ww])
                    nc.vector.tensor_copy(dte[:, :ww], dts[:, :ww])
                    nc.sync.dma_start(out=dbg_eT[:, j0: j0 + ww], in_=dte[:, :ww])

            # ============ layers ============
            for l in range(L):
                # ---- node phase: hd = silu(h @ Wdown + bdown) -> allgather
                for w0 in range(0, NW, 4):
                    nwin = min(4, NW - w0)
                    hdt = sb.tile([P, 4, F], dt.bfloat16, tag="hd4")
                    for a in range(nwin):
                        w = w0 + a
                        php = sps.tile([P, F], dt.float32, tag="sps")
                        nc.tensor.matmul(php[:], lhsT=c_ones1[:],
                                         rhs=c_bdown1[:, l, :], start=True, stop=False)
                        nc.tensor.matmul(php[:], lhsT=hTb[:, w * P:(w + 1) * P],
                                         rhs=c_Wdown[:, l, :], start=False, stop=True)
                        act_silu(hdt[:, a, :], php[:])
                    nc.sync.dma_start(
                        out=hd_local[:].rearrange("(a p) d -> p a d", p=P)[
                            :, w0: w0 + nwin, :],
                        in_=hdt[:, :nwin, :])
                nc.gpsimd.collective_compute(
                    "AllGather", AL.bypass,
                    replica_groups=[list(range(NC_))],
                    ins=[hd_local[:].opt()], outs=[hd_full[:].opt()])

                # ---- edge phase ----
                agg_open = {}
                for call_i, (base_ch, ncall, half) in enumerate(calls):
                    gt = gat.tile([P, GCALL, F], dt.bfloat16, tag="hdg")
                    ni = ncall * P
                    nc.gpsimd.dma_gather(
                        gt[:, :ncall, :],
                        hd_full[half * (NC_ // 2) * NSHARD:, :],
                        c_idx[:, base_ch * 8: base_ch * 8 + ni // 16],
                        ni, ni, F, queue_num=call_i % 4)
                    if l == 0:
                        # produce eT a couple of calls ahead of consumption
                        ensure_embed((base_ch + 3 * GCALL) * P + ni)
                    eTt = wpool.tile([P, GCALL, F], dt.bfloat16, tag="eTt", bufs=6)
                    nc.sync.dma_start(
                        out=eTt[:, :ncall, :],
                        in_=eT_dram[:, base_ch * P: base_ch * P + ni].rearrange(
                            "p (c q) -> p c q", q=P))
                    for s0 in range(0, ncall, WSUB):
                        nsub = min(WSUB, ncall - s0)
                        wp = mps.tile([P, WSUB, F], dt.float32, tag="mps")
                        nc.tensor.matmul(
                            wp[:, :nsub, :].rearrange("p a q -> p (a q)"),
                            lhsT=c_ones1[:],
                            rhs=c_bgeom8[:, l, : nsub * F],
                            start=True, stop=False)
                        for ci in range(nsub):
                            nc.tensor.matmul(
                                wp[:, ci, :], lhsT=eTt[:, s0 + ci, :],
                                rhs=c_Wgeom[:, l, :], start=False, stop=(ci == nsub - 1))
                        wsb = wpool.tile([P, WSUB, F], dt.bfloat16, tag="wsb", bufs=16)
                        act_silu(wsb[:, :nsub, :], wp[:, :nsub, :])
                        msg = sb.tile([P, WSUB, F], dt.bfloat16, tag="msg")
                        nc.vector.tensor_mul(msg[:, :nsub, :], wsb[:, :nsub, :],
                                             gt[:, s0: s0 + nsub, :])
                        oh = sb.tile([P, WSUB, F], dt.bfloat16, tag="oh")
                        nc.vector.tensor_tensor(
                            out=oh[:, :nsub, :],
                            in0=c_iota8[:, :nsub, :],
                            in1=c_dstloc[:, base_ch + s0: base_ch + s0 + nsub
                                         ].to_broadcast([P, nsub, P]),
                            op=AL.is_equal)
                        for ci in range(nsub):
                            ch = base_ch + s0 + ci
                            w = int(chunk_win[ch])
                            in_lo = ch < NLOC
                            q = QLO if in_lo else QHI
                            rel_c = ch - (0 if in_lo else NLOC)
                            first = rel_c % q == 0
                            last = rel_c % q == q - 1
                            key = (w, in_lo)
                            if first:
                                agg_open[key] = aggps.tile(
                                    [P, F], dt.float32, tag="aggps",
                                    name=f"aggp_{l}_{ch}")
                            nc.tensor.matmul(agg_open[key][:], lhsT=oh[:, ci, :],
                                             rhs=msg[:, ci, :],
                                             start=first, stop=last)
                            if last:
                                if in_lo:
                                    nc.vector.tensor_copy(agg_sb[:, w, :],
                                                          agg_open[key][:])
                                else:
                                    nc.vector.tensor_add(agg_sb[:, w, :],
                                                         agg_sb[:, w, :],
                                                         agg_open[key][:])
                                del agg_open[key]

                if debug and l == 0:
                    for w in range(NW):
                        dta = sb.tile([P, F], dt.float32, name=f"da{w}", tag="dbg")
                        nc.vector.tensor_copy(dta[:], agg_sb[:, w, :])
                        nc.sync.dma_start(
                            out=dbg_agg[:, w, :], in_=dta[:])
                    for w in range(NW):
                        dhs = sb.tile([P, F], dt.bfloat16, name=f"dq{w}", tag="dbg2")
                        nc.sync.dma_start(
                            out=dhs[:],
                            in_=hd_local[:].rearrange("(a p) d -> p a d", p=P)[:, w, :])
                        dth = sb.tile([P, F], dt.float32, name=f"dh{w}", tag="dbg")
                        nc.vector.tensor_copy(dth[:], dhs[:])
                        nc.sync.dma_start(
                            out=dbg_hd.rearrange("(a p) d -> p a d", p=P)[:, w, :],
                            in_=dth[:])

                # ---- GraphNorm + update ----
                gsum_p = gps.tile([GPC, H], dt.float32, tag="gps")
                for w in range(NW):
                    nc.tensor.matmul(gsum_p[:], lhsT=c_boh[:, w, :],
                                     rhs=agg_sb[:, w, :],
                                     start=(w == 0), stop=(w == NW - 1))
                tmean = sb2.tile([GPC, H], dt.float32, tag="gn32")
                nc.vector.tensor_scalar(out=tmean[:], in0=gsum_p[:],
                                        scalar1=c_cnt_inv[:], scalar2=None,
                                        op0=AL.mult)
                mean_sc = sb2.tile([GPC, H], dt.bfloat16, tag="gn")
                nc.vector.tensor_mul(mean_sc[:], tmean[:], c_gnmsB[:, l, :])
                for w in range(NW):
                    mb = sps.tile([P, H], dt.float32, tag="sps")
                    nc.tensor.matmul(mb[:], lhsT=c_bohT[:, w, :], rhs=mean_sc[:],
                                     start=True, stop=True)
                    nc.vector.tensor_sub(ctr_sb[:, w, :], agg_sb[:, w, :], mb[:])
                sq_p = gps.tile([GPC, H], dt.float32, tag="gps")
                for w in range(NW):
                    sq = sb2.tile([P, H], dt.bfloat16, tag="sq")
                    nc.vector.tensor_mul(sq[:], ctr_sb[:, w, :], ctr_sb[:, w, :])
                    nc.tensor.matmul(sq_p[:], lhsT=c_boh[:, w, :], rhs=sq[:],
                                     start=(w == 0), stop=(w == NW - 1))
                var = sb2.tile([GPC, H], dt.float32, tag="gn32")
                nc.vector.tensor_scalar(out=var[:], in0=sq_p[:],
                                        scalar1=c_cnt_inv[:], scalar2=None,
                                        op0=AL.mult)
                sd = sb2.tile([GPC, H], dt.float32, tag="gn32")
                nc.scalar.activation(sd[:], var[:], SQT, bias=c_eps[:])
                rs = sb2.tile([GPC, H], dt.float32, tag="gn32")
                nc.vector.reciprocal(rs[:], sd[:])
                scale = sb2.tile([GPC, H], dt.bfloat16, tag="gn")
                nc.vector.tensor_mul(scale[:], rs[:], c_gnwB[:, l, :])
                for w0 in range(0, NW, 4):
                    nwin = min(4, NW - w0)
                    hnT4 = sb2.tile([F, 4 * P], dt.bfloat16, tag="hnT4")
                    for a in range(nwin):
                        w = w0 + a
                        sbp = sps.tile([P, H], dt.float32, tag="sps")
                        nc.tensor.matmul(sbp[:], lhsT=c_bohT[:, w, :], rhs=scale[:],
                                         start=True, stop=True)
                        hn = sb2.tile([P, H], dt.float32, tag="hn")
                        nc.vector.tensor_mul(hn[:], ctr_sb[:, w, :], sbp[:])
                        hn2 = sb2.tile([P, H], dt.float32, tag="hn2")
                        nc.vector.tensor_add(hn2[:], hn[:], c_gnbB[:, l, :])
                        shn = sb2.tile([P, H], dt.bfloat16, tag="shn")
                        act_silu(shn[:], hn2[:])
                        tp = sps.tile([P, P], dt.bfloat16, tag="sps")
                        nc.tensor.transpose(tp[:], shn[:], c_ident[:])
                        nc.vector.tensor_copy(hnT4[:, a * P:(a + 1) * P], tp[:])
                    upp = mps.tile([H, 4 * P], dt.float32, tag="mps")
                    nc.tensor.matmul(upp[:, : nwin * P], lhsT=c_Wup[:, l, :],
                                     rhs=hnT4[:, : nwin * P], start=True, stop=True)
                    ups = sb2.tile([H, 4 * P], dt.float32, tag="ups")
                    act_silu(ups[:, : nwin * P], upp[:, : nwin * P], bias=c_bup[:, l: l + 1])
                    nc.vector.tensor_add(hT[:, w0 * P: w0 * P + nwin * P],
                                         hT[:, w0 * P: w0 * P + nwin * P],
                                         ups[:, : nwin * P])
                    nc.vector.tensor_copy(hTb[:, w0 * P: w0 * P + nwin * P],
                                          hT[:, w0 * P: w0 * P + nwin * P])

                if debug and l == 0:
                    for j0 in range(0, NSHARD, TN):
                        w_ = min(TN, NSHARD - j0)
                        dt1 = sb.tile([H, TN], dt.float32, name=f"d1{j0}", tag="dbg")
                        nc.vector.tensor_copy(dt1[:, :w_], hT[:, j0: j0 + w_])
                        nc.sync.dma_start(out=dbg_h1[:, j0: j0 + w_], in_=dt1[:, :w_])

            # ============ output block ============
            z_p = gps.tile([GPC, 64], dt.float32, tag="gps")
            for w in range(NW):
                t3p = sps.tile([P, 64], dt.float32, tag="sps")
                nc.tensor.matmul(t3p[:], lhsT=c_ones1[:], rhs=c_bo11[:],
                                 start=True, stop=False)
                nc.tensor.matmul(t3p[:], lhsT=hTb[:, w * P:(w + 1) * P],
                                 rhs=c_Wo1[:], start=False, stop=True)
                t3 = sb2.tile([P, 64], dt.bfloat16, tag="t3b")
                act_silu(t3[:], t3p[:])
                nc.tensor.matmul(z_p[:], lhsT=c_boh[:, w, :], rhs=t3[:],
                                 start=(w == 0), stop=(w == NW - 1))
            z_sb = sb2.tile([GPC, 64], dt.bfloat16, tag="zsb")
            nc.vector.tensor_copy(z_sb[:], z_p[:])
            zT_p = sps.tile([64, GPC], dt.bfloat16, tag="sps")
            nc.tensor.transpose(zT_p[:], z_sb[:], c_ident[:GPC, :GPC])
            zT = sb2.tile([64, GPC], dt.bfloat16, tag="zT")
            nc.vector.tensor_copy(zT[:], zT_p[:])
            en_p = sps.tile([GPC, 1], dt.float32, tag="sps")
            nc.tensor.matmul(en_p[:], lhsT=zT[:], rhs=c_Wo2[:],
                             start=True, stop=True)
            en = sb2.tile([GPC, 1], dt.float32, tag="en")
            nc.vector.tensor_add(en[:], en_p[:], c_cntbo2[:])
            nc.sync.dma_start(out=energy[:], in_=en[:])

    nc.compile()
    return nc


def _install_ntff_hook():
    """Restore antenv.axon_hooks + register the ctypes NTFF hook."""
    import types

    try:
        from antenv.axon_hooks import get_axon_ntff_profile_hook  # noqa: F401

        return
    except ImportError:
        pass
    try:
        import antenv

        mod = types.ModuleType("antenv.axon_hooks")
        mod._hook = None

        def _set(h):
            mod._hook = h

        def _get():
            return mod._hook

        mod.set_axon_ntff_profile_hook = _set
        mod.get_axon_ntff_profile_hook = _get
        sys.modules["antenv.axon_hooks"] = mod
        antenv.axon_hooks = mod
        sys.path.insert(0, "/root/.axon_site")
        from trn_agent_boot.trn_boot import _ntff_profile_via_ctypes

        hook = _ntff_profile_via_ctypes("/opt/axon/libaxon_pjrt.so")
        if hook is not None:
            _set(hook)
    except Exception as e:  # pragma: no cover
        print(f"ntff hook install failed: {e}", file=sys.stderr)


def kernel(**inputs) -> np.ndarray:
    meta, in_maps, _extra = _build_host(inputs)
    nc = _build_program(meta)
    from concourse.bass_utils import run_bass_kernel_spmd

    trace = bool(int(os.environ.get("KERNEL_TRACE", "0")))
    if trace:
        _install_ntff_hook()
    res = run_bass_kernel_spmd(nc, in_maps, core_ids=list(range(NC_)), trace=trace)
    if trace:
        kernel.last_results = res
    out = np.concatenate([res.results[k]["energy"] for k in range(NC_)], axis=0)
    return out.astype(np.float32)

